# revision 1
# baseline (speedup 1.0000x reference)
"""Trainium2 Bass kernel for nn_CrossAttention (B=2, C=512, N=M=2048, H=8).

Sharding: batch*heads = 16 (b,h) pairs across 8 cores, 2 heads per core.
Cores 0-3 handle batch 0 (heads in pairs), cores 4-7 batch 1.

Per-core pipeline (bf16 compute, fp32 PSUM accumulation):
  kT[d,m] = Wk_cols.T @ y_b          (2 heads packed on partitions)
  qT[d,n] = (Wq_cols * SCALE).T @ x_b
  v2[m, 1+d | 1+d] = y_blk.T @ Wv'   (direct [m,d] layout, ones cols preset;
                                      Wv' has the depthwise conv folded in)
  S^T[m,n] = kT_h.T-slices @ qT_h    (row-packed K=64 pairs per head)
  P = exp(S^T) -> bf16               (ScalarE streaming [128,1024] blocks)
  acc[n, 1+d] += P_blk.T @ v2[m]     (flipped attnout: P is the stationary,
                                      65-wide free -> 65 cyc/matmul; col 0 of
                                      each group accumulates the denominator;
                                      one start/stop per PSUM bank since the
                                      start bit zeroes the whole 2KB bank)
  nrm[n, d] = acc * recip(den)       (DVE per-partition scalar multiply)
  attnT[c, n] = PE-transpose(nrm)    (bf16)
  outT_partial[cout, n] = Wp_rows.T @ attnT   (bf16 partials to HBM)

Host folds (1+lw) into Wv, bias' = bp + lb @ Wp (exact: softmax rows sum
to 1), sums the 4 per-batch partials, adds bias'.
"""

import os
import sys
import numpy as np
from contextlib import ExitStack

for _p in ("/root/.axon_site", "/root/.axon_site/_ro/trn_rl_repo",
           "/root/.axon_site/_ro/pypackages", "/opt/trn_rl_repo"):
    if os.path.isdir(_p) and _p not in sys.path:
        sys.path.append(_p)

B, C, N, M, H = 2, 512, 2048, 2048, 8
HD = C // H
SCALE = HD ** -0.5
NCORES = 8

_NC = None
LAST_RUN = None


DEFAULT_CFG = dict(warm=7, ppool=5, v2pro=4, m0split=True, defer=True,
                   lag=3, hold=3, tail="streams", tpc_act=True, dma2=True,
                   so_act=False, post0=2, m1early=False, divide=False,
                   divide_tail=False, v2shift=0, ktq=False, defer_m=14, v2tail=1, qoff=1,
                   postpack=1)


def _build_program(reps=1, cfg=None):
    cfg = dict(DEFAULT_CFG, **(cfg or {}))
    # the previous accumulator must be copied out (post0) only after its
    # last attnout has been emitted, which happens at step lag-1
    cfg["post0"] = max(cfg["post0"], cfg["lag"])
    from concourse import bacc
    import concourse.tile as tile
    import concourse.mybir as mybir
    from concourse.masks import make_identity

    F32 = mybir.dt.float32
    BF16 = mybir.dt.bfloat16
    EXP = mybir.ActivationFunctionType.Exp
    COPY = mybir.ActivationFunctionType.Copy
    MULT = mybir.AluOpType.mult
    DIV = mybir.AluOpType.divide

    nc = bacc.Bacc("TRN2", target_bir_lowering=False, debug=False,
                   num_devices=NCORES)

    xr = nc.dram_tensor("xr", [C, N], BF16, kind="ExternalInput").ap()
    yr = nc.dram_tensor("yr", [C, M], BF16, kind="ExternalInput").ap()
    # wall = [Wk' | Wq' | Wv'] concatenated so one DMA loads all three
    wall_d = nc.dram_tensor("wall", [C, 384], BF16, kind="ExternalInput").ap()
    wp_d = nc.dram_tensor("wp", [128, C], BF16, kind="ExternalInput").ap()
    outT = nc.dram_tensor("outT", [C, N], BF16, kind="ExternalOutput").ap()

    xr4 = xr.rearrange("(kc p) n -> p kc n", p=128)
    yr4 = yr.rearrange("(kc p) n -> p kc n", p=128)
    outT4 = outT.rearrange("(cc p) n -> p cc n", p=128)

    with tile.TileContext(nc) as tc, ExitStack() as ctx:
        sb = ctx.enter_context(tc.tile_pool(name="sb", bufs=1))
        ppool = ctx.enter_context(tc.tile_pool(name="ppool", bufs=cfg["ppool"]))
        npool = ctx.enter_context(tc.tile_pool(name="npool", bufs=2))
        spool = ctx.enter_context(tc.tile_pool(name="spool", bufs=2))
        # PSUM budget (8 banks): psA ring 3x[128,1024]f32 = 6 banks (scores,
        # proj/v2 staging, transposes, outproj transients); psB 2x1 bank
        # (attnout accumulators; the tail reuses them for outproj).
        psA = ctx.enter_context(tc.tile_pool(name="psA", bufs=3, space="PSUM"))
        psB = ctx.enter_context(tc.tile_pool(name="psB", bufs=2, space="PSUM"))

        # ---- PE warm-up with no DMA dependency: DVE-zeroed operand ----
        zwarm = sb.tile([128, 128], F32, tag="zwarm")
        nc.vector.memset(zwarm, 0.0)
        # warm the exp table while DMAs stream
        warm = sb.tile([1, 32], F32, tag="warm")
        nc.scalar.activation(warm, zwarm[0:1, 0:32], EXP)
        psw = psA.tile([128, 128], F32, tag="blk", name="psw")
        for _ in range(cfg["warm"]):
            nc.tensor.matmul(psw, zwarm, zwarm, start=True, stop=True)
        warm2 = sb.tile([128, 128], F32, tag="warm2")
        nc.vector.tensor_copy(warm2, psw)

        # ---- input DMAs, all on the sync-engine HWDGE queue; order is
        # the first-use order so the global DMA serialization helps the
        # prologue rather than hurting it ----
        wall_sb = sb.tile([128, 4, 384], BF16, tag="wall_sb")
        wp_sb = sb.tile([128, C], BF16, tag="wp_sb")
        y_sb = sb.tile([128, 4, M], BF16, tag="y_sb")
        x_sb = sb.tile([128, 4, N], BF16, tag="x_sb")
        wk_sb = wall_sb[:, :, 0:128]
        wq_sb = wall_sb[:, :, 128:256]
        wv_sb = wall_sb[:, :, 256:384]

        def load_j(dst, src, j):
            nc.sync.dma_start(out=dst[:, :, j * 512:(j + 1) * 512],
                              in_=src[:, :, j * 512:(j + 1) * 512])

        def load_half(dst, src, h):
            nc.sync.dma_start(out=dst[:, :, h * 256:(h + 1) * 256],
                              in_=src[:, :, h * 256:(h + 1) * 256])

        wall4 = wall_d.rearrange("(kc p) m -> p kc m", p=128)
        # wk+wq first (gates both first projections), then the j0 halves
        # interleaved y/x so the first score piece's deps land earliest
        nc.sync.dma_start(out=wall_sb[:, :, 0:256], in_=wall4[:, :, 0:256])
        load_half(y_sb, yr4, 0)
        load_half(x_sb, xr4, 0)
        load_half(y_sb, yr4, 1)
        load_half(x_sb, xr4, 1)
        nc.sync.dma_start(out=wall_sb[:, :, 256:384], in_=wall4[:, :, 256:384])
        load_j(y_sb, yr4, 1)
        load_j(x_sb, xr4, 1)
        nc.sync.dma_start(out=wp_sb, in_=wp_d)
        load_j(y_sb, yr4, 2)
        load_j(x_sb, xr4, 2)
        load_j(y_sb, yr4, 3)
        load_j(x_sb, xr4, 3)

        # identity (bf16, for PE transposes) built on the idle GPSIMD
        identb = sb.tile([128, 128], BF16, tag="identb")
        make_identity(nc, identb)

        # v2[p, mb, col]: col 0 = ones (head a den), 1..64 = head a values,
        # col 65 = ones (head b den), 66..129 = head b values.
        v2 = sb.tile([128, 16, 130], BF16, tag="v2")
        nc.vector.memset(v2[:, :, 0:1], 1.0)
        nc.vector.memset(v2[:, :, 65:66], 1.0)

        kT = sb.tile([128, M], BF16, tag="kT")
        qT = sb.tile([128, N], BF16, tag="qT")

        hold = {}

        def proj_half(dst, w_sb, src, j, half, name):
            if half == 0:
                hold[name] = psA.tile([128, 512], F32, tag="blk", name=name)
            ps = hold[name]
            for kc in (0, 1) if half == 0 else (2, 3):
                nc.tensor.matmul(ps, w_sb[:, kc, :],
                                 src[:, kc, j * 512:(j + 1) * 512],
                                 start=(kc == 0), stop=(kc == 3))
            if half == 1:
                nc.vector.tensor_copy(dst[:, j * 512:(j + 1) * 512], ps)

        def v2_task(mb):
            ps = psA.tile([128, 128], F32, tag="blk", name=f"v2ps{mb}")
            ms = slice(mb * 128, (mb + 1) * 128)
            for kc in range(4):
                nc.tensor.matmul(ps, y_sb[:, kc, ms], wv_sb[:, kc, :],
                                 start=(kc == 0), stop=(kc == 3))
            nc.vector.tensor_copy(v2[:, mb, 1:65], ps[:, 0:64])
            nc.vector.tensor_copy(v2[:, mb, 66:130], ps[:, 64:128])

        def proj256(dst, w_sb, src, h, name):
            # 256-wide projection so work starts at half-DMA arrival
            ps = psA.tile([128, 256], F32, tag="blk", name=name)
            sl = slice(h * 256, (h + 1) * 256)
            for kc in range(4):
                nc.tensor.matmul(ps, w_sb[:, kc, :], src[:, kc, sl],
                                 start=(kc == 0), stop=(kc == 3))
            nc.vector.tensor_copy(dst[:, sl], ps)

        # ---- prologue: only what gates scores(chunk0, m=0..3); the qT
        # copies go ahead of the v2 copies on the DVE queue since the
        # first exp gates on qT while v2 is only needed two steps in ----
        P00 = None
        if cfg["m0split"]:
            P00 = ppool.tile([128, 1024], BF16, tag="p", name="p0_0")

        def m0_piece(h):
            # tile_position'd matmuls may not share a PSUM bank: head
            # pieces go to separate banks, exp reads them strided.
            nh = slice(h * 256, (h + 1) * 256)
            blk = psA.tile([128, 2, 512], F32, tag="blk", name=f"blk0_0{h}")
            nc.tensor.matmul(blk[:, 0, 0:256], kT[0:64, 0:128],
                             qT[0:64, nh], start=True, stop=True,
                             tile_position=(0, 0))
            nc.tensor.matmul(blk[:, 1, 0:256], kT[64:128, 0:128],
                             qT[64:128, nh], start=True, stop=True,
                             tile_position=(64, 0))
            nc.scalar.activation(P00[:, h * 512:(h + 1) * 512],
                                 blk[:, :, 0:256], EXP)

        proj256(kT, wk_sb, y_sb, 0, "psk0a")
        proj256(qT, wq_sb, x_sb, 0, "psq0a")
        if cfg["m0split"]:
            m0_piece(0)
        proj256(kT, wk_sb, y_sb, 1, "psk0b")
        proj256(qT, wq_sb, x_sb, 1, "psq0b")
        if cfg["m0split"]:
            m0_piece(1)
        P01 = None
        if cfg["m1early"]:
            # m=1 scores+exp also ahead of the v2 prologue block so the
            # exp stream has no gap while v2 builds
            blk01 = psA.tile([128, 1024], F32, tag="blk", name="blk0_1")
            nc.tensor.matmul(blk01[:, 0:512], kT[0:64, 128:256],
                             qT[0:64, 0:512], start=True, stop=True,
                             tile_position=(0, 0))
            nc.tensor.matmul(blk01[:, 512:1024], kT[64:128, 128:256],
                             qT[64:128, 0:512], start=True, stop=True,
                             tile_position=(64, 0))
            P01 = ppool.tile([128, 1024], BF16, tag="p", name="p0_1")
            nc.scalar.activation(P01, blk01, EXP)
        # early v2 blocks ride the PE while later DMAs are in flight
        for _mb in range(cfg["v2pro"]):
            v2_task(_mb)

        # weave tasks: late projections + v2 blocks. Emission order defines
        # data dependencies (a consumer emitted before its producer reads
        # stale SBUF), so each task must be emitted strictly before its
        # first consumer: kT j before scores(0, 4j), v2(mb) before the
        # attnout(mb) emission (mb+2), qT j before scores(j, 0).
        def P_(dst, w, src, j, half, name):
            return lambda: proj_half(dst, w, src, j, half, name)

        def proj_q(dst, w_sb, src, j, kc, name):
            if kc == 0:
                hold[name] = psA.tile([128, 512], F32, tag="blk", name=name)
            ps = hold[name]
            nc.tensor.matmul(ps, w_sb[:, kc, :],
                             src[:, kc, j * 512:(j + 1) * 512],
                             start=(kc == 0), stop=(kc == 3))
            if kc == 3:
                nc.vector.tensor_copy(dst[:, j * 512:(j + 1) * 512], ps)

        def Q_(dst, w, src, j, kc, name):
            return lambda: proj_q(dst, w, src, j, kc, name)

        _v2rem = list(range(cfg["v2pro"], 4))
        _v2h = (len(_v2rem) + 1) // 2
        _vs = cfg["v2shift"]
        fills = {
            (0, 0): [(lambda mb=mb: v2_task(mb)) for mb in _v2rem[:_v2h]],
            (0, 1): [(lambda mb=mb: v2_task(mb)) for mb in _v2rem[_v2h:]],
            (0, 2): [P_(kT, wk_sb, y_sb, 1, 0, "psk1")],
            (0, 3): [P_(kT, wk_sb, y_sb, 1, 1, "psk1")],
        }
        _qo = cfg["qoff"]
        fills[(1, _qo)] = [P_(qT, wq_sb, x_sb, 2, 0, "psq2")]
        fills[(1, _qo + 1)] = [P_(qT, wq_sb, x_sb, 2, 1, "psq2")]
        fills[(2, _qo)] = [P_(qT, wq_sb, x_sb, 3, 0, "psq3")]
        fills[(2, _qo + 1)] = [P_(qT, wq_sb, x_sb, 3, 1, "psq3")]
        if cfg["ktq"]:
            # quarter-granular late projections: one matmul per weave slot
            for i, (j, kc) in enumerate([(2, 0), (2, 1), (2, 2), (2, 3),
                                         (3, 0), (3, 1), (3, 2), (3, 3)]):
                fills.setdefault((0, 4 + i), []).append(
                    Q_(kT, wk_sb, y_sb, j, kc, f"psk{j}"))
            for kc in range(4):
                fills.setdefault((0, 12 + kc), []).append(
                    Q_(qT, wq_sb, x_sb, 1, kc, "psq1"))
        else:
            fills[(0, 6)] = [P_(kT, wk_sb, y_sb, 2, 0, "psk2")]
            fills[(0, 7)] = [P_(kT, wk_sb, y_sb, 2, 1, "psk2")]
            fills[(0, 10)] = [P_(kT, wk_sb, y_sb, 3, 0, "psk3")]
            fills[(0, 11)] = [P_(kT, wk_sb, y_sb, 3, 1, "psk3")]
            fills[(0, 13)] = [P_(qT, wq_sb, x_sb, 1, 0, "psq1")]
            fills[(0, 14)] = [P_(qT, wq_sb, x_sb, 1, 1, "psq1")]
        for mb in range(4, 16):
            # v2(mb) must be emitted before attnout(mb) pops at step
            # mb+lag (which may fall in chunk 1); shift within that
            # window to smooth the chunk-0 PE load
            step = mb + _vs + (cfg["v2tail"] if mb >= 14 else 0)
            key = (0, step) if step <= 15 else (1, step - 16)
            fills.setdefault(key, []).append(lambda mb=mb: v2_task(mb))

        # ---- main attention loop ----
        pending = []         # (P, acc_a, acc_b, m) awaiting attnout
        post = []            # deferred post-processing closures

        def emit_attnout(P, acc_a, acc_b, m):
            # PSUM zero-region = one full 2KB bank: exactly one start
            # (m=0,s=0) and one stop (m=15,s=3) per accumulator tile.
            for s in range(4):
                if isinstance(P, tuple):  # split first m-step: [a256|b256]x2
                    Pt = P[1]
                    h, i = s // 2, s % 2
                    pa = Pt[:, h * 512 + i * 128:h * 512 + (i + 1) * 128]
                    pb = Pt[:, h * 512 + 256 + i * 128:
                            h * 512 + 256 + (i + 1) * 128]
                else:
                    pa = P[:, s * 128:(s + 1) * 128]
                    pb = P[:, 512 + s * 128:512 + (s + 1) * 128]
                nc.tensor.matmul(acc_a[:, s, :], pa, v2[:, m, 0:65],
                                 start=(m == 0 and s == 0),
                                 stop=(m == 15 and s == 3))
                nc.tensor.matmul(acc_b[:, s, :], pb, v2[:, m, 65:130],
                                 start=(m == 0 and s == 0),
                                 stop=(m == 15 and s == 3))

        def make_post(c, acc_a, acc_b):
            st = {}

            def grab(aps, dst_tag, out_name):
                # one fast PSUM->SBUF copy releases the accumulator bank
                t = spool.tile([128, 4, 65], F32, tag=dst_tag, name=out_name)
                nc.vector.tensor_copy(t, aps)
                return t

            def grab_a():
                st["ca"] = grab(acc_a, "ca", f"ca{c}")
                st["cb"] = grab(acc_b, "cb", f"cb{c}")

            def recips():
                st["attnT"] = npool.tile([128, 512], BF16, tag="attnT",
                                         name=f"attnT{c}")
                if not cfg["divide"]:
                    st["ra"] = spool.tile([128, 4], F32, tag="ra",
                                          name=f"ra{c}")
                    st["rb"] = spool.tile([128, 4], F32, tag="rb",
                                          name=f"rb{c}")
                    nc.vector.reciprocal(st["ra"], st["ca"][:, :, 0:1])
                    nc.vector.reciprocal(st["rb"], st["cb"][:, :, 0:1])

            def sub(s):
                trin = spool.tile([128, 128], BF16, tag=f"trin{s % 2}",
                                  name=f"trin{c}_{s}")
                if cfg["divide"]:
                    nc.vector.tensor_scalar(trin[:, 0:64],
                                            st["ca"][:, s, 1:65],
                                            st["ca"][:, s, 0:1], None,
                                            op0=DIV)
                    nc.vector.tensor_scalar(trin[:, 64:128],
                                            st["cb"][:, s, 1:65],
                                            st["cb"][:, s, 0:1], None,
                                            op0=DIV)
                else:
                    nc.vector.tensor_scalar(trin[:, 0:64],
                                            st["ca"][:, s, 1:65],
                                            st["ra"][:, s:s + 1], None,
                                            op0=MULT)
                    nc.vector.tensor_scalar(trin[:, 64:128],
                                            st["cb"][:, s, 1:65],
                                            st["rb"][:, s:s + 1], None,
                                            op0=MULT)
                tp = psA.tile([128, 128], BF16, tag="blk", name=f"tp{c}_{s}")
                nc.tensor.transpose(tp, trin, identb)
                nc.vector.tensor_copy(
                    st["attnT"][:, s * 128:(s + 1) * 128], tp)

            def outproj(cc):
                po = psA.tile([128, 512], F32, tag="blk", name=f"po{c}_{cc}")
                nc.tensor.matmul(po, wp_sb[:, cc * 128:(cc + 1) * 128],
                                 st["attnT"], start=True, stop=True)
                if cc == 0:
                    st["so"] = npool.tile([128, 4, 512], BF16, tag="so",
                                          name=f"so{c}")
                if cfg["so_act"] and cc % 2 == 1:
                    nc.scalar.copy(st["so"][:, cc, :], po)
                else:
                    nc.vector.tensor_copy(st["so"][:, cc, :], po)
                if cc == 3:
                    nc.sync.dma_start(
                        out=outT4[:, :, c * 512:(c + 1) * 512],
                        in_=st["so"])

            tasks = ([[grab_a], [recips]]
                     + [[lambda s=s: sub(s)] for s in range(4)]
                     + [[lambda cc=cc: outproj(cc)] for cc in range(4)])
            if cfg["postpack"] > 1:
                k = cfg["postpack"]
                tasks = [sum(tasks[i:i + k], []) for i in range(0, 10, k)]
            return tasks

        for c in range(4):
            ns = slice(c * 512, (c + 1) * 512)
            acc_a = psB.tile([128, 4, 65], F32, tag="acc", name=f"acca{c}")
            acc_b = psB.tile([128, 4, 65], F32, tag="acc", name=f"accb{c}")
            for m in range(16):
                ms = slice(m * 128, (m + 1) * 128)
                if cfg["m0split"] and c == 0 and m == 0:
                    # scores+exp for m0 were already emitted in the
                    # prologue (split into two x-half-gated pieces with
                    # layout [a n256 | b n256] per half)
                    P = ("split", P00)
                elif cfg["m1early"] and c == 0 and m == 1:
                    P = P01
                else:
                    blk = psA.tile([128, 1024], F32, tag="blk",
                                   name=f"blk{c}_{m}")
                    nc.tensor.matmul(blk[:, 0:512], kT[0:64, ms],
                                     qT[0:64, ns],
                                     start=True, stop=True,
                                     tile_position=(0, 0))
                    nc.tensor.matmul(blk[:, 512:1024], kT[64:128, ms],
                                     qT[64:128, ns],
                                     start=True, stop=True,
                                     tile_position=(64, 0))
                    P = ppool.tile([128, 1024], BF16, tag="p",
                                   name=f"p{c}_{m}")
                    nc.scalar.activation(P, blk, EXP)
                if post and m >= cfg["post0"]:
                    for task in post.pop(0):
                        task()
                for task in fills.pop((c, m), ()):
                    task()
                pending.append((P, acc_a, acc_b, m))
                # lag 2 normally; a new chunk's first attnout (which waits
                # for the previous accumulator bank to be copied out by
                # grab_a) is held until m=3 so it never stalls the in-order
                # PE queue ahead of the score stream; the last two steps of
                # the last chunk defer entirely so the final exps are not
                # delayed behind attnout matmuls.
                while len(pending) > cfg["lag"] and not (
                        c > 0 and m < cfg["hold"] and
                        pending[0][3] == 0) and not (
                        cfg["defer"] and c == 3 and m >= cfg["defer_m"]):
                    emit_attnout(*pending.pop(0))
            post = make_post(c, acc_a, acc_b)

        # ---- drain + tail (chunk 3 post-processing, pipelined) ----
        # Read the accumulators straight from PSUM (no ring pressure at
        # the end), split normalize/copy work across DVE and the now-idle
        # ScalarE (Copy shares the exp activation table, no reload), and
        # run the output projection per n-sub-block so copies and DMAs
        # start as early as possible. Two po tiles reuse the accumulator
        # banks freed at the start of the tail.
        while pending:
            emit_attnout(*pending.pop(0))
        c = 3
        if not cfg["divide_tail"]:
            ra = spool.tile([128, 4], F32, tag="ra", name="ra3")
            rb = spool.tile([128, 4], F32, tag="rb", name="rb3")
            nc.vector.reciprocal(ra, acc_a[:, :, 0:1])
            nc.vector.reciprocal(rb, acc_b[:, :, 0:1])
        attnT3 = npool.tile([128, 512], BF16, tag="attnT", name="attnT3")
        trins = [spool.tile([128, 128], BF16, tag="ttr", name=f"trin3_{s}",
                            bufs=4) for s in range(4)]
        # per-engine streams with no cross-engine ping-pong: DVE runs all
        # normalizes then the transpose copies; PE runs transposes then
        # the per-sub output projections; ScalarE+DVE split the output
        # copies; two DMAs so the first half ships early.
        po = [psB.tile([128, 512], F32, tag="acc", name="po3_0"),
              psB.tile([128, 512], F32, tag="acc", name="po3_1"),
              psA.tile([128, 512], F32, tag="blk", name="po3_2"),
              psA.tile([128, 512], F32, tag="blk", name="po3_3")]
        # tail scheduling variants, tuned against the timeline sim
        def t_ts(s):
            if cfg["divide_tail"]:
                nc.vector.tensor_scalar(trins[s][:, 0:64], acc_a[:, s, 1:65],
                                        acc_a[:, s, 0:1], None, op0=DIV)
                nc.vector.tensor_scalar(trins[s][:, 64:128],
                                        acc_b[:, s, 1:65],
                                        acc_b[:, s, 0:1], None, op0=DIV)
            else:
                nc.vector.tensor_scalar(trins[s][:, 0:64], acc_a[:, s, 1:65],
                                        ra[:, s:s + 1], None, op0=MULT)
                nc.vector.tensor_scalar(trins[s][:, 64:128],
                                        acc_b[:, s, 1:65],
                                        rb[:, s:s + 1], None, op0=MULT)

        def t_tp(s):
            tp = psA.tile([128, 128], BF16, tag="blk", name=f"tp3_{s}")
            nc.tensor.transpose(tp, trins[s], identb)
            if cfg["tpc_act"]:
                nc.scalar.copy(attnT3[:, s * 128:(s + 1) * 128], tp)
            else:
                nc.vector.tensor_copy(attnT3[:, s * 128:(s + 1) * 128], tp)

        def t_po(s):
            for cc in range(4):
                nc.tensor.matmul(po[cc][:, s * 128:(s + 1) * 128],
                                 wp_sb[:, cc * 128:(cc + 1) * 128],
                                 attnT3[:, s * 128:(s + 1) * 128],
                                 start=(s == 0), stop=(s == 3))

        if cfg["tail"] == "streams":
            for s in range(4):
                t_ts(s)
            for s in range(4):
                t_tp(s)
            for s in range(4):
                t_po(s)
        else:
            for s in range(4):
                t_ts(s)
                t_tp(s)
                t_po(s)
        so3 = npool.tile([128, 4, 512], BF16, tag="so", name="so3")
        nc.vector.tensor_copy(so3[:, 0, :], po[0])
        nc.scalar.copy(so3[:, 1, :], po[1])
        if cfg["dma2"]:
            nc.sync.dma_start(out=outT4[:, 0:2, c * 512:(c + 1) * 512],
                              in_=so3[:, 0:2, :])
        nc.vector.tensor_copy(so3[:, 2, :], po[2])
        nc.scalar.copy(so3[:, 3, :], po[3])
        if cfg["dma2"]:
            nc.sync.dma_start(out=outT4[:, 2:4, c * 512:(c + 1) * 512],
                              in_=so3[:, 2:4, :])
        else:
            nc.sync.dma_start(out=outT4[:, :, c * 512:(c + 1) * 512],
                              in_=so3)

    nc.compile()
    return nc


def _get_program():
    global _NC
    if _NC is None:
        _NC = _build_program()
    return _NC


def make_in_maps(inputs):
    import ml_dtypes
    bf16 = ml_dtypes.bfloat16

    x = np.asarray(inputs["x"], np.float32)
    y = np.asarray(inputs["y"], np.float32)
    Wq = np.asarray(inputs["Wq"], np.float32)
    Wkv = np.asarray(inputs["Wkv"], np.float32)
    lw = np.asarray(inputs["lw"], np.float32)
    Wp = np.asarray(inputs["Wp"], np.float32)

    d = np.arange(HD)
    xb = [np.ascontiguousarray(x[b]).astype(bf16) for b in range(B)]
    yb = [np.ascontiguousarray(y[b]).astype(bf16) for b in range(B)]
    in_maps = []
    for core in range(NCORES):
        b = core // 4
        h0 = (core % 4) * 2
        ch = np.concatenate([h * HD + d for h in (h0, h0 + 1)])  # channels
        colsK = np.concatenate([h * 2 * HD + 2 * d for h in (h0, h0 + 1)])
        wq_c = Wq[:, ch] * np.float32(SCALE)
        wk_c = Wkv[:, colsK]
        wv_c = Wkv[:, colsK + 1] * (1.0 + lw[ch])[None, :]
        wp_c = Wp[ch, :]
        wall = np.concatenate([wk_c, wq_c, wv_c], axis=1)  # [C, 384]
        in_maps.append({
            "xr": xb[b],
            "yr": yb[b],
            "wall": np.ascontiguousarray(wall).astype(bf16),
            "wp": np.ascontiguousarray(wp_c).astype(bf16),
        })
    return in_maps


def assemble_output(results, inputs):
    lb = np.asarray(inputs["lb"], np.float32)
    Wp = np.asarray(inputs["Wp"], np.float32)
    bp = np.asarray(inputs["bp"], np.float32)
    bias = (bp + lb @ Wp).astype(np.float32)
    parts = [np.asarray(results[i]["outT"], dtype=np.float32)
             for i in range(NCORES)]
    out = np.stack([
        parts[0] + parts[1] + parts[2] + parts[3],
        parts[4] + parts[5] + parts[6] + parts[7],
    ])
    out += bias[None, :, None]
    return out.astype(np.float32)


def kernel(x, y, Wq, Wkv, lw, lb, Wp, bp):
    global LAST_RUN
    from concourse.bass_utils import run_bass_kernel_spmd

    inputs = dict(x=x, y=y, Wq=Wq, Wkv=Wkv, lw=lw, lb=lb, Wp=Wp, bp=bp)
    nc = _get_program()
    in_maps = make_in_maps(inputs)
    LAST_RUN = run_bass_kernel_spmd(nc, in_maps, list(range(NCORES)))
    return assemble_output(LAST_RUN.results, inputs)



# revision 32
# speedup vs baseline: 1.0491x; 1.0491x over previous
"""Trainium2 Bass kernel for nn_CrossAttention (B=2, C=512, N=M=2048, H=8).

Sharding: batch*heads = 16 (b,h) pairs across 8 cores, 2 heads per core.
Cores 0-3 handle batch 0 (heads in pairs), cores 4-7 batch 1.

Per-core pipeline (bf16 compute, fp32 PSUM accumulation):
  kT[d,m] = Wk_cols.T @ y_b          (2 heads packed on partitions)
  qT[d,n] = (Wq_cols * SCALE).T @ x_b
  v2[m, 1+d | 1+d] = y_blk.T @ Wv'   (direct [m,d] layout, ones cols preset;
                                      Wv' has the depthwise conv folded in)
  S^T[m,n] = kT_h.T-slices @ qT_h    (row-packed K=64 pairs per head)
  P = exp(S^T) -> bf16               (ScalarE streaming [128,1024] blocks)
  acc[n, 1+d] += P_blk.T @ v2[m]     (flipped attnout: P is the stationary,
                                      65-wide free -> 65 cyc/matmul; col 0 of
                                      each group accumulates the denominator;
                                      one start/stop per PSUM bank since the
                                      start bit zeroes the whole 2KB bank)
  nrm[n, d] = acc * recip(den)       (DVE per-partition scalar multiply)
  attnT[c, n] = PE-transpose(nrm)    (bf16)
  outT_partial[cout, n] = Wp_rows.T @ attnT   (bf16 partials to HBM)

Host folds (1+lw) into Wv, bias' = bp + lb @ Wp (exact: softmax rows sum
to 1), sums the 4 per-batch partials, adds bias'.
"""

import os
import sys
import numpy as np
from contextlib import ExitStack

for _p in ("/root/.axon_site", "/root/.axon_site/_ro/trn_rl_repo",
           "/root/.axon_site/_ro/pypackages", "/opt/trn_rl_repo"):
    if os.path.isdir(_p) and _p not in sys.path:
        sys.path.append(_p)

B, C, N, M, H = 2, 512, 2048, 2048, 8
HD = C // H
SCALE = HD ** -0.5
NCORES = 8

_NC = None
LAST_RUN = None

# ---- custom DVE exp: exp(32*y) = poly4(y)^32 for y in [-0.4, 0.4] ----
# Offloads part of the softmax exp stream from the (bottleneck) Scalar
# engine to the Vector engine. Scores are pre-scaled by 1/32 (folded into
# Wq); ScalarE blocks use activation(..., scale=32) at no extra cost.
EXP_C0 = 0.5001447017887652
EXP_C1 = 0.16771833562855049
EXP_C2 = 0.04088734265090471

_EXP_OPS = {}


def _register_exp_ops():
    import numpy as np
    from concourse import dve_ops
    from concourse.dve_spec import Spec, Src0, C0, C1, C2, One, lower
    from concourse.dve_uop import DveOpSpec
    from concourse.dve_ops import DveOp, _SUB_OPCODE_FOR_NAME

    if _EXP_OPS:
        return _EXP_OPS

    _y = Src0
    _s = _y * _y
    poly_body = ((C2 * _s + C1 * _y + C0) * _s + _y) + One
    _q = Src0
    _q2 = _q * _q
    _q4 = _q2 * _q2
    _q8 = _q4 * _q4
    _q16 = _q8 * _q8
    sq32_body = _q16 * _q16

    def ref_poly(in0, in1, s0, s1, imm2):
        y = in0.astype(np.float32)
        s = (y * y).astype(np.float32)
        u = (np.float32(imm2) * s).astype(np.float32)
        v = (np.float32(s1) * y).astype(np.float32)
        w = ((u + v).astype(np.float32) + np.float32(s0)).astype(np.float32)
        z = ((w * s).astype(np.float32) + y).astype(np.float32)
        return (z + np.float32(1.0)).astype(np.float32)

    def ref_sq32(in0, in1, s0, s1, imm2):
        q = in0.astype(np.float32)
        for _ in range(5):
            q = (q * q).astype(np.float32)
        return q

    def make_op(name, body, ref):
        spec = Spec(body=body, reference=ref)
        tmp = DveOpSpec(name=name, opcode=0, uops=lower(spec, ver="v3"),
                        rd1_en=False)
        return DveOp(name, spec, subdim=False, uops_sha={"v3": tmp.sha("v3")})

    for name, body, ref in (("EXP_POLY_ANT", poly_body, ref_poly),
                            ("EXP_SQ32_ANT", sq32_body, ref_sq32)):
        if name not in _SUB_OPCODE_FOR_NAME:
            op = make_op(name, body, ref)
            row = max(_SUB_OPCODE_FOR_NAME.values()) + 1
            assert row < 0x20
            dve_ops.OPS.append(op)
            _SUB_OPCODE_FOR_NAME[name] = row
            dve_ops.CUSTOM_DVE_SPECS[name] = op.spec
        else:
            op = next(o for o in dve_ops.OPS if o.name == name)
        _EXP_OPS[name] = op
    return _EXP_OPS


DEFAULT_CFG = dict(warm=7, ppool=5, v2pro=4, m0split=True, defer=True,
                   lag=3, hold=3, tail="streams", tpc_act=True, dma2=True,
                   so_act=False, post0=2, m1early=False, divide=False,
                   divide_tail=False, v2shift=0, ktq=False, defer_m=14, v2tail=1, qoff=1,
                   postpack=1, dmaorder=True, v2pair=True,
                   post_steps=(3, 4, 8, 9, 10, 11, 12, 13, 14, 15),
                   dvexp={(1, 1): 1024, (1, 6): 1024,
                          (2, 1): 1024, (2, 6): 1024,
                          (3, 1): 1024, (3, 6): 1024})


def _build_program(reps=1, cfg=None):
    cfg = dict(DEFAULT_CFG, **(cfg or {}))
    # the previous accumulator must be copied out (post0) only after its
    # last attnout has been emitted, which happens at step lag-1
    cfg["post0"] = max(cfg["post0"], cfg["lag"])
    exp_ops = _register_exp_ops()
    EXP_POLY = exp_ops["EXP_POLY_ANT"]
    EXP_SQ32 = exp_ops["EXP_SQ32_ANT"]
    dvexp = dict(cfg["dvexp"])
    from concourse import bacc
    import concourse.tile as tile
    import concourse.mybir as mybir
    from concourse.masks import make_identity

    F32 = mybir.dt.float32
    BF16 = mybir.dt.bfloat16
    EXP = mybir.ActivationFunctionType.Exp
    COPY = mybir.ActivationFunctionType.Copy
    MULT = mybir.AluOpType.mult
    DIV = mybir.AluOpType.divide

    nc = bacc.Bacc("TRN2", target_bir_lowering=False, debug=False,
                   num_devices=NCORES)

    xr = nc.dram_tensor("xr", [C, N], BF16, kind="ExternalInput").ap()
    yr = nc.dram_tensor("yr", [C, M], BF16, kind="ExternalInput").ap()
    # wall = [Wk' | Wq' | Wv'] concatenated so one DMA loads all three
    wall_d = nc.dram_tensor("wall", [C, 384], BF16, kind="ExternalInput").ap()
    wp_d = nc.dram_tensor("wp", [128, C], BF16, kind="ExternalInput").ap()
    outT = nc.dram_tensor("outT", [C, N], BF16, kind="ExternalOutput").ap()

    xr4 = xr.rearrange("(kc p) n -> p kc n", p=128)
    yr4 = yr.rearrange("(kc p) n -> p kc n", p=128)
    outT4 = outT.rearrange("(cc p) n -> p cc n", p=128)

    with tile.TileContext(nc) as tc, ExitStack() as ctx:
        sb = ctx.enter_context(tc.tile_pool(name="sb", bufs=1))
        ppool = ctx.enter_context(tc.tile_pool(name="ppool", bufs=cfg["ppool"]))
        qpool = ctx.enter_context(tc.tile_pool(name="qpool", bufs=2))
        npool = ctx.enter_context(tc.tile_pool(name="npool", bufs=2))
        spool = ctx.enter_context(tc.tile_pool(name="spool", bufs=2))
        # PSUM budget (8 banks): psA ring 3x[128,1024]f32 = 6 banks (scores,
        # proj/v2 staging, transposes, outproj transients); psB 2x1 bank
        # (attnout accumulators; the tail reuses them for outproj).
        psA = ctx.enter_context(tc.tile_pool(name="psA", bufs=3, space="PSUM"))
        psB = ctx.enter_context(tc.tile_pool(name="psB", bufs=2, space="PSUM"))

        # ---- PE warm-up with no DMA dependency: DVE-zeroed operand ----
        zwarm = sb.tile([128, 128], F32, tag="zwarm")
        nc.vector.memset(zwarm, 0.0)
        # warm the exp table while DMAs stream
        warm = sb.tile([1, 32], F32, tag="warm")
        nc.scalar.activation(warm, zwarm[0:1, 0:32], EXP)
        psw = psA.tile([128, 128], F32, tag="blk", name="psw")
        for _ in range(cfg["warm"]):
            nc.tensor.matmul(psw, zwarm, zwarm, start=True, stop=True)
        warm2 = sb.tile([128, 128], F32, tag="warm2")
        nc.vector.tensor_copy(warm2, psw)

        # ---- input DMAs, all on the sync-engine HWDGE queue; order is
        # the first-use order so the global DMA serialization helps the
        # prologue rather than hurting it ----
        wall_sb = sb.tile([128, 4, 384], BF16, tag="wall_sb")
        wp_sb = sb.tile([128, C], BF16, tag="wp_sb")
        y_sb = sb.tile([128, 4, M], BF16, tag="y_sb")
        x_sb = sb.tile([128, 4, N], BF16, tag="x_sb")
        wk_sb = wall_sb[:, :, 0:128]
        wq_sb = wall_sb[:, :, 128:256]
        wv_sb = wall_sb[:, :, 256:384]

        def load_j(dst, src, j):
            nc.sync.dma_start(out=dst[:, :, j * 512:(j + 1) * 512],
                              in_=src[:, :, j * 512:(j + 1) * 512])

        def load_half(dst, src, h):
            nc.sync.dma_start(out=dst[:, :, h * 256:(h + 1) * 256],
                              in_=src[:, :, h * 256:(h + 1) * 256])

        wall4 = wall_d.rearrange("(kc p) m -> p kc m", p=128)
        # wk+wq first (gates both first projections), then the j0 halves;
        # x_h1 before y_h1: qT h1 (x cols 256:512) gates the m0 second
        # piece and m1 scores, while y_h1 (kT h1 = m-blocks 2,3) is only
        # needed for scores m2+.
        nc.sync.dma_start(out=wall_sb[:, :, 0:256], in_=wall4[:, :, 0:256])
        load_half(y_sb, yr4, 0)
        load_half(x_sb, xr4, 0)
        if cfg["dmaorder"]:
            load_half(x_sb, xr4, 1)
            load_half(y_sb, yr4, 1)
        else:
            load_half(y_sb, yr4, 1)
            load_half(x_sb, xr4, 1)
        nc.sync.dma_start(out=wall_sb[:, :, 256:384], in_=wall4[:, :, 256:384])
        load_j(y_sb, yr4, 1)
        load_j(x_sb, xr4, 1)
        nc.sync.dma_start(out=wp_sb, in_=wp_d)
        load_j(y_sb, yr4, 2)
        load_j(x_sb, xr4, 2)
        load_j(y_sb, yr4, 3)
        load_j(x_sb, xr4, 3)

        # identity (bf16, for PE transposes) built on the idle GPSIMD
        identb = sb.tile([128, 128], BF16, tag="identb")
        make_identity(nc, identb)

        # v2[p, mb, col]: col 0 = ones (head a den), 1..64 = head a values,
        # col 65 = ones (head b den), 66..129 = head b values.
        v2 = sb.tile([128, 16, 130], BF16, tag="v2")
        nc.vector.memset(v2[:, :, 0:1], 1.0)
        nc.vector.memset(v2[:, :, 65:66], 1.0)

        kT = sb.tile([128, M], BF16, tag="kT")
        qT = sb.tile([128, N], BF16, tag="qT")

        hold = {}

        def proj_half(dst, w_sb, src, j, half, name):
            if half == 0:
                hold[name] = psA.tile([128, 512], F32, tag="blk", name=name)
            ps = hold[name]
            for kc in (0, 1) if half == 0 else (2, 3):
                nc.tensor.matmul(ps, w_sb[:, kc, :],
                                 src[:, kc, j * 512:(j + 1) * 512],
                                 start=(kc == 0), stop=(kc == 3))
            if half == 1:
                nc.vector.tensor_copy(dst[:, j * 512:(j + 1) * 512], ps)

        def v2_task(mb):
            ps = psA.tile([128, 128], F32, tag="blk", name=f"v2ps{mb}")
            ms = slice(mb * 128, (mb + 1) * 128)
            for kc in range(4):
                nc.tensor.matmul(ps, y_sb[:, kc, ms], wv_sb[:, kc, :],
                                 start=(kc == 0), stop=(kc == 3))
            # one strided copy for both head groups: [128,(2,64)] view
            nc.vector.tensor_copy(
                v2[:, mb, :].rearrange("p (g c) -> p g c", g=2)[:, :, 1:65],
                ps.rearrange("p (g c) -> p g c", g=2))

        def v2_task2(mb):
            # two m-blocks in ONE psA allocation (single bank): keeps the
            # score-ring alternation intact where v2 and proj fills would
            # otherwise put two allocations between consecutive score blocks
            ps = psA.tile([128, 256], F32, tag="blk", name=f"v2ps{mb}")
            for k, b in enumerate((mb, mb + 1)):
                msl = slice(b * 128, (b + 1) * 128)
                for kc in range(4):
                    nc.tensor.matmul(ps[:, k * 128:(k + 1) * 128],
                                     y_sb[:, kc, msl], wv_sb[:, kc, :],
                                     start=(b == mb and kc == 0),
                                     stop=(b == mb + 1 and kc == 3))
            for k, b in enumerate((mb, mb + 1)):
                nc.vector.tensor_copy(
                    v2[:, b, :].rearrange("p (g c) -> p g c", g=2)[:, :, 1:65],
                    ps[:, k * 128:(k + 1) * 128].rearrange(
                        "p (g c) -> p g c", g=2))

        def proj256(dst, w_sb, src, h, name):
            # 256-wide projection so work starts at half-DMA arrival
            ps = psA.tile([128, 256], F32, tag="blk", name=name)
            sl = slice(h * 256, (h + 1) * 256)
            for kc in range(4):
                nc.tensor.matmul(ps, w_sb[:, kc, :], src[:, kc, sl],
                                 start=(kc == 0), stop=(kc == 3))
            nc.vector.tensor_copy(dst[:, sl], ps)

        # ---- prologue: only what gates scores(chunk0, m=0..3); the qT
        # copies go ahead of the v2 copies on the DVE queue since the
        # first exp gates on qT while v2 is only needed two steps in ----
        P00 = None
        if cfg["m0split"]:
            P00 = ppool.tile([128, 1024], BF16, tag="p", name="p0_0")

        def m0_piece(h):
            # tile_position'd matmuls may not share a PSUM bank: head
            # pieces go to separate banks, exp reads them strided.
            nh = slice(h * 256, (h + 1) * 256)
            blk = psA.tile([128, 2, 512], F32, tag="blk", name=f"blk0_0{h}")
            nc.tensor.matmul(blk[:, 0, 0:256], kT[0:64, 0:128],
                             qT[0:64, nh], start=True, stop=True,
                             tile_position=(0, 0))
            nc.tensor.matmul(blk[:, 1, 0:256], kT[64:128, 0:128],
                             qT[64:128, nh], start=True, stop=True,
                             tile_position=(64, 0))
            nc.scalar.activation(P00[:, h * 512:(h + 1) * 512],
                                 blk[:, :, 0:256], EXP, scale=32.0)

        proj256(kT, wk_sb, y_sb, 0, "psk0a")
        proj256(qT, wq_sb, x_sb, 0, "psq0a")
        if cfg["m0split"]:
            m0_piece(0)
        proj256(kT, wk_sb, y_sb, 1, "psk0b")
        proj256(qT, wq_sb, x_sb, 1, "psq0b")
        if cfg["m0split"]:
            m0_piece(1)
        P01 = None
        if cfg["m1early"]:
            # m=1 scores+exp also ahead of the v2 prologue block so the
            # exp stream has no gap while v2 builds
            blk01 = psA.tile([128, 1024], F32, tag="blk", name="blk0_1")
            nc.tensor.matmul(blk01[:, 0:512], kT[0:64, 128:256],
                             qT[0:64, 0:512], start=True, stop=True,
                             tile_position=(0, 0))
            nc.tensor.matmul(blk01[:, 512:1024], kT[64:128, 128:256],
                             qT[64:128, 0:512], start=True, stop=True,
                             tile_position=(64, 0))
            P01 = ppool.tile([128, 1024], BF16, tag="p", name="p0_1")
            nc.scalar.activation(P01, blk01, EXP, scale=32.0)
        # early v2 blocks ride the PE while later DMAs are in flight
        for _mb in range(cfg["v2pro"]):
            v2_task(_mb)

        # weave tasks: late projections + v2 blocks. Emission order defines
        # data dependencies (a consumer emitted before its producer reads
        # stale SBUF), so each task must be emitted strictly before its
        # first consumer: kT j before scores(0, 4j), v2(mb) before the
        # attnout(mb) emission (mb+2), qT j before scores(j, 0).
        def P_(dst, w, src, j, half, name):
            return lambda: proj_half(dst, w, src, j, half, name)

        def proj_q(dst, w_sb, src, j, kc, name):
            if kc == 0:
                hold[name] = psA.tile([128, 512], F32, tag="blk", name=name)
            ps = hold[name]
            nc.tensor.matmul(ps, w_sb[:, kc, :],
                             src[:, kc, j * 512:(j + 1) * 512],
                             start=(kc == 0), stop=(kc == 3))
            if kc == 3:
                nc.vector.tensor_copy(dst[:, j * 512:(j + 1) * 512], ps)

        def Q_(dst, w, src, j, kc, name):
            return lambda: proj_q(dst, w, src, j, kc, name)

        _v2rem = list(range(cfg["v2pro"], 4))
        _v2h = (len(_v2rem) + 1) // 2
        _vs = cfg["v2shift"]
        fills = {
            (0, 0): [(lambda mb=mb: v2_task(mb)) for mb in _v2rem[:_v2h]],
            (0, 1): [(lambda mb=mb: v2_task(mb)) for mb in _v2rem[_v2h:]],
            (0, 2): [P_(kT, wk_sb, y_sb, 1, 0, "psk1")],
            (0, 3): [P_(kT, wk_sb, y_sb, 1, 1, "psk1")],
        }
        _qo = cfg["qoff"]
        fills[(1, _qo)] = [P_(qT, wq_sb, x_sb, 2, 0, "psq2")]
        fills[(1, _qo + 1)] = [P_(qT, wq_sb, x_sb, 2, 1, "psq2")]
        fills[(2, _qo)] = [P_(qT, wq_sb, x_sb, 3, 0, "psq3")]
        fills[(2, _qo + 1)] = [P_(qT, wq_sb, x_sb, 3, 1, "psq3")]
        if cfg["ktq"]:
            # quarter-granular late projections: one matmul per weave slot
            for i, (j, kc) in enumerate([(2, 0), (2, 1), (2, 2), (2, 3),
                                         (3, 0), (3, 1), (3, 2), (3, 3)]):
                fills.setdefault((0, 4 + i), []).append(
                    Q_(kT, wk_sb, y_sb, j, kc, f"psk{j}"))
            for kc in range(4):
                fills.setdefault((0, 12 + kc), []).append(
                    Q_(qT, wq_sb, x_sb, 1, kc, "psq1"))
        else:
            fills[(0, 6)] = [P_(kT, wk_sb, y_sb, 2, 0, "psk2")]
            fills[(0, 7)] = [P_(kT, wk_sb, y_sb, 2, 1, "psk2")]
            fills[(0, 10)] = [P_(kT, wk_sb, y_sb, 3, 0, "psk3")]
            fills[(0, 11)] = [P_(kT, wk_sb, y_sb, 3, 1, "psk3")]
            fills[(0, 13)] = [P_(qT, wq_sb, x_sb, 1, 0, "psq1")]
            fills[(0, 14)] = [P_(qT, wq_sb, x_sb, 1, 1, "psq1")]
        if cfg["v2pair"]:
            # pairs on proj-free steps; each pair's deadline is the attnout
            # pop of its first block (step mb+lag)
            for mb, step in ((4, 4), (6, 5), (8, 8), (10, 9), (12, 12),
                             (14, 15)):
                fills.setdefault((0, step), []).append(
                    lambda mb=mb: v2_task2(mb))
        else:
            for mb in range(4, 16):
                # v2(mb) must be emitted before attnout(mb) pops at step
                # mb+lag (which may fall in chunk 1); shift within that
                # window to smooth the chunk-0 PE load
                step = mb + _vs + (cfg["v2tail"] if mb >= 14 else 0)
                key = (0, step) if step <= 15 else (1, step - 16)
                fills.setdefault(key, []).append(lambda mb=mb: v2_task(mb))

        # ---- main attention loop ----
        pending = []         # (P, acc_a, acc_b, m) awaiting attnout
        post = []            # deferred post-processing closures

        def emit_attnout(P, acc_a, acc_b, m):
            # PSUM zero-region = one full 2KB bank: exactly one start
            # (m=0,s=0) and one stop (m=15,s=3) per accumulator tile.
            for s in range(4):
                if isinstance(P, tuple):  # split first m-step: [a256|b256]x2
                    Pt = P[1]
                    h, i = s // 2, s % 2
                    pa = Pt[:, h * 512 + i * 128:h * 512 + (i + 1) * 128]
                    pb = Pt[:, h * 512 + 256 + i * 128:
                            h * 512 + 256 + (i + 1) * 128]
                else:
                    pa = P[:, s * 128:(s + 1) * 128]
                    pb = P[:, 512 + s * 128:512 + (s + 1) * 128]
                nc.tensor.matmul(acc_a[:, s, :], pa, v2[:, m, 0:65],
                                 start=(m == 0 and s == 0),
                                 stop=(m == 15 and s == 3))
                nc.tensor.matmul(acc_b[:, s, :], pb, v2[:, m, 65:130],
                                 start=(m == 0 and s == 0),
                                 stop=(m == 15 and s == 3))

        def make_post(c, acc_a, acc_b):
            st = {}

            def grab(aps, dst_tag, out_name):
                # one fast PSUM->SBUF copy releases the accumulator bank
                t = spool.tile([128, 4, 65], F32, tag=dst_tag, name=out_name)
                nc.vector.tensor_copy(t, aps)
                return t

            def grab_a():
                st["ca"] = grab(acc_a, "ca", f"ca{c}")
                st["cb"] = grab(acc_b, "cb", f"cb{c}")

            def recips():
                st["attnT"] = npool.tile([128, 512], BF16, tag="attnT",
                                         name=f"attnT{c}")
                if not cfg["divide"]:
                    st["ra"] = spool.tile([128, 4], F32, tag="ra",
                                          name=f"ra{c}")
                    st["rb"] = spool.tile([128, 4], F32, tag="rb",
                                          name=f"rb{c}")
                    nc.vector.reciprocal(st["ra"], st["ca"][:, :, 0:1])
                    nc.vector.reciprocal(st["rb"], st["cb"][:, :, 0:1])

            def sub(s):
                trin = spool.tile([128, 128], BF16, tag=f"trin{s % 2}",
                                  name=f"trin{c}_{s}")
                if cfg["divide"]:
                    nc.vector.tensor_scalar(trin[:, 0:64],
                                            st["ca"][:, s, 1:65],
                                            st["ca"][:, s, 0:1], None,
                                            op0=DIV)
                    nc.vector.tensor_scalar(trin[:, 64:128],
                                            st["cb"][:, s, 1:65],
                                            st["cb"][:, s, 0:1], None,
                                            op0=DIV)
                else:
                    nc.vector.tensor_scalar(trin[:, 0:64],
                                            st["ca"][:, s, 1:65],
                                            st["ra"][:, s:s + 1], None,
                                            op0=MULT)
                    nc.vector.tensor_scalar(trin[:, 64:128],
                                            st["cb"][:, s, 1:65],
                                            st["rb"][:, s:s + 1], None,
                                            op0=MULT)
                tp = psA.tile([128, 128], BF16, tag="blk", name=f"tp{c}_{s}")
                nc.tensor.transpose(tp, trin, identb)
                nc.vector.tensor_copy(
                    st["attnT"][:, s * 128:(s + 1) * 128], tp)

            def outproj(cc):
                po = psA.tile([128, 512], F32, tag="blk", name=f"po{c}_{cc}")
                nc.tensor.matmul(po, wp_sb[:, cc * 128:(cc + 1) * 128],
                                 st["attnT"], start=True, stop=True)
                if cc == 0:
                    st["so"] = npool.tile([128, 4, 512], BF16, tag="so",
                                          name=f"so{c}")
                if cfg["so_act"] and cc % 2 == 1:
                    nc.scalar.copy(st["so"][:, cc, :], po)
                else:
                    nc.vector.tensor_copy(st["so"][:, cc, :], po)
                if cc == 3:
                    nc.sync.dma_start(
                        out=outT4[:, :, c * 512:(c + 1) * 512],
                        in_=st["so"])

            tasks = ([[grab_a], [recips]]
                     + [[lambda s=s: sub(s)] for s in range(4)]
                     + [[lambda cc=cc: outproj(cc)] for cc in range(4)])
            if cfg["postpack"] > 1:
                k = cfg["postpack"]
                tasks = [sum(tasks[i:i + k], []) for i in range(0, 10, k)]
            return tasks

        for c in range(4):
            ns = slice(c * 512, (c + 1) * 512)
            acc_a = psB.tile([128, 4, 65], F32, tag="acc", name=f"acca{c}")
            acc_b = psB.tile([128, 4, 65], F32, tag="acc", name=f"accb{c}")
            sched = None
            if cfg["post_steps"] is not None and post:
                sched = {}
                for i, grp in enumerate(post):
                    sched.setdefault(cfg["post_steps"][i], []).append(grp)
                post = []
            for m in range(16):
                ms = slice(m * 128, (m + 1) * 128)
                if cfg["m0split"] and c == 0 and m == 0:
                    # scores+exp for m0 were already emitted in the
                    # prologue (split into two x-half-gated pieces with
                    # layout [a n256 | b n256] per half)
                    P = ("split", P00)
                elif cfg["m1early"] and c == 0 and m == 1:
                    P = P01
                else:
                    blk = psA.tile([128, 1024], F32, tag="blk",
                                   name=f"blk{c}_{m}")
                    nc.tensor.matmul(blk[:, 0:512], kT[0:64, ms],
                                     qT[0:64, ns],
                                     start=True, stop=True,
                                     tile_position=(0, 0))
                    nc.tensor.matmul(blk[:, 512:1024], kT[64:128, ms],
                                     qT[64:128, ns],
                                     start=True, stop=True,
                                     tile_position=(64, 0))
                    P = ppool.tile([128, 1024], BF16, tag="p",
                                   name=f"p{c}_{m}")
                    dn = dvexp.get((c, m), 0)
                    if dn:
                        # split the exp: ScalarE takes cols [0:1024-dn],
                        # DVE (poly + 5 squarings) takes the tail [1024-dn:]
                        x0 = 1024 - dn
                        if x0:
                            nc.scalar.activation(P[:, 0:x0], blk[:, 0:x0],
                                                 EXP, scale=32.0)
                        q = qpool.tile([128, dn], F32, tag="q",
                                       name=f"q{c}_{m}")
                        nc.vector._custom_dve(EXP_POLY, out=q,
                                              in0=blk[:, x0:1024],
                                              s0=EXP_C0, s1=EXP_C1,
                                              imm2=EXP_C2)
                        nc.vector._custom_dve(EXP_SQ32, out=P[:, x0:1024],
                                              in0=q)
                    else:
                        nc.scalar.activation(P, blk, EXP, scale=32.0)
                if sched is not None:
                    for grp in sched.pop(m, ()):
                        for task in grp:
                            task()
                elif post and m >= cfg["post0"]:
                    for task in post.pop(0):
                        task()
                for task in fills.pop((c, m), ()):
                    task()
                pending.append((P, acc_a, acc_b, m))
                # lag 2 normally; a new chunk's first attnout (which waits
                # for the previous accumulator bank to be copied out by
                # grab_a) is held until m=3 so it never stalls the in-order
                # PE queue ahead of the score stream; the last two steps of
                # the last chunk defer entirely so the final exps are not
                # delayed behind attnout matmuls.
                while len(pending) > cfg["lag"] and not (
                        c > 0 and m < cfg["hold"] and
                        pending[0][3] == 0) and not (
                        cfg["defer"] and c == 3 and m >= cfg["defer_m"]):
                    emit_attnout(*pending.pop(0))
            post = make_post(c, acc_a, acc_b)

        # ---- drain + tail (chunk 3 post-processing, pipelined) ----
        # Read the accumulators straight from PSUM (no ring pressure at
        # the end), split normalize/copy work across DVE and the now-idle
        # ScalarE (Copy shares the exp activation table, no reload), and
        # run the output projection per n-sub-block so copies and DMAs
        # start as early as possible. Two po tiles reuse the accumulator
        # banks freed at the start of the tail.
        while pending:
            emit_attnout(*pending.pop(0))
        c = 3
        if not cfg["divide_tail"]:
            ra = spool.tile([128, 4], F32, tag="ra", name="ra3")
            rb = spool.tile([128, 4], F32, tag="rb", name="rb3")
            nc.vector.reciprocal(ra, acc_a[:, :, 0:1])
            nc.vector.reciprocal(rb, acc_b[:, :, 0:1])
        attnT3 = npool.tile([128, 512], BF16, tag="attnT", name="attnT3")
        trins = [spool.tile([128, 128], BF16, tag="ttr", name=f"trin3_{s}",
                            bufs=4) for s in range(4)]
        # per-engine streams with no cross-engine ping-pong: DVE runs all
        # normalizes then the transpose copies; PE runs transposes then
        # the per-sub output projections; ScalarE+DVE split the output
        # copies; two DMAs so the first half ships early.
        po = [psB.tile([128, 512], F32, tag="acc", name="po3_0"),
              psB.tile([128, 512], F32, tag="acc", name="po3_1"),
              psA.tile([128, 512], F32, tag="blk", name="po3_2"),
              psA.tile([128, 512], F32, tag="blk", name="po3_3")]
        # tail scheduling variants, tuned against the timeline sim
        def t_ts(s):
            if cfg["divide_tail"]:
                nc.vector.tensor_scalar(trins[s][:, 0:64], acc_a[:, s, 1:65],
                                        acc_a[:, s, 0:1], None, op0=DIV)
                nc.vector.tensor_scalar(trins[s][:, 64:128],
                                        acc_b[:, s, 1:65],
                                        acc_b[:, s, 0:1], None, op0=DIV)
            else:
                nc.vector.tensor_scalar(trins[s][:, 0:64], acc_a[:, s, 1:65],
                                        ra[:, s:s + 1], None, op0=MULT)
                nc.vector.tensor_scalar(trins[s][:, 64:128],
                                        acc_b[:, s, 1:65],
                                        rb[:, s:s + 1], None, op0=MULT)

        def t_tp(s):
            tp = psA.tile([128, 128], BF16, tag="blk", name=f"tp3_{s}")
            nc.tensor.transpose(tp, trins[s], identb)
            if cfg["tpc_act"]:
                nc.scalar.copy(attnT3[:, s * 128:(s + 1) * 128], tp)
            else:
                nc.vector.tensor_copy(attnT3[:, s * 128:(s + 1) * 128], tp)

        def t_po(s):
            for cc in range(4):
                nc.tensor.matmul(po[cc][:, s * 128:(s + 1) * 128],
                                 wp_sb[:, cc * 128:(cc + 1) * 128],
                                 attnT3[:, s * 128:(s + 1) * 128],
                                 start=(s == 0), stop=(s == 3))

        if cfg["tail"] == "v2":
            # normalize split across DVE (head a) and ScalarE (head b, Copy
            # with per-partition scale AP); outproj cc-major so each po bank
            # completes right after the transposes; per-cc quarter DMAs.
            for s in range(4):
                nc.vector.tensor_scalar(trins[s][:, 0:64], acc_a[:, s, 1:65],
                                        ra[:, s:s + 1], None, op0=MULT)
                nc.scalar.mul(trins[s][:, 64:128], acc_b[:, s, 1:65],
                              rb[:, s:s + 1])
            for s in range(4):
                tp = psA.tile([128, 128], BF16, tag="blk", name=f"tp3_{s}")
                nc.tensor.transpose(tp, trins[s], identb)
                if s % 2 == 0:
                    nc.scalar.copy(attnT3[:, s * 128:(s + 1) * 128], tp)
                else:
                    nc.vector.tensor_copy(attnT3[:, s * 128:(s + 1) * 128],
                                          tp)
            so3 = npool.tile([128, 4, 512], BF16, tag="so", name="so3")
            for s in range(4):
                t_po(s)
            for cc in range(4):
                if cc % 2 == 0:
                    nc.vector.tensor_copy(so3[:, cc, :], po[cc])
                else:
                    nc.scalar.copy(so3[:, cc, :], po[cc])
                nc.sync.dma_start(
                    out=outT4[:, cc:cc + 1, c * 512:(c + 1) * 512],
                    in_=so3[:, cc:cc + 1, :])
        else:
            if cfg["tail"] == "streams":
                for s in range(4):
                    t_ts(s)
                for s in range(4):
                    t_tp(s)
                for s in range(4):
                    t_po(s)
            else:
                for s in range(4):
                    t_ts(s)
                    t_tp(s)
                    t_po(s)
            so3 = npool.tile([128, 4, 512], BF16, tag="so", name="so3")
            nc.vector.tensor_copy(so3[:, 0, :], po[0])
            nc.scalar.copy(so3[:, 1, :], po[1])
            if cfg["dma2"]:
                nc.sync.dma_start(out=outT4[:, 0:2, c * 512:(c + 1) * 512],
                                  in_=so3[:, 0:2, :])
            nc.vector.tensor_copy(so3[:, 2, :], po[2])
            nc.scalar.copy(so3[:, 3, :], po[3])
            if cfg["dma2"]:
                nc.sync.dma_start(out=outT4[:, 2:4, c * 512:(c + 1) * 512],
                                  in_=so3[:, 2:4, :])
            else:
                nc.sync.dma_start(out=outT4[:, :, c * 512:(c + 1) * 512],
                                  in_=so3)

    nc.compile()
    return nc


def _get_program():
    global _NC
    if _NC is None:
        _NC = _build_program()
    return _NC


def make_in_maps(inputs):
    import ml_dtypes
    bf16 = ml_dtypes.bfloat16

    x = np.asarray(inputs["x"], np.float32)
    y = np.asarray(inputs["y"], np.float32)
    Wq = np.asarray(inputs["Wq"], np.float32)
    Wkv = np.asarray(inputs["Wkv"], np.float32)
    lw = np.asarray(inputs["lw"], np.float32)
    Wp = np.asarray(inputs["Wp"], np.float32)

    d = np.arange(HD)
    xb = [np.ascontiguousarray(x[b]).astype(bf16) for b in range(B)]
    yb = [np.ascontiguousarray(y[b]).astype(bf16) for b in range(B)]
    in_maps = []
    for core in range(NCORES):
        b = core // 4
        h0 = (core % 4) * 2
        ch = np.concatenate([h * HD + d for h in (h0, h0 + 1)])  # channels
        colsK = np.concatenate([h * 2 * HD + 2 * d for h in (h0, h0 + 1)])
        wq_c = Wq[:, ch] * np.float32(SCALE / 32.0)
        wk_c = Wkv[:, colsK]
        wv_c = Wkv[:, colsK + 1] * (1.0 + lw[ch])[None, :]
        wp_c = Wp[ch, :]
        wall = np.concatenate([wk_c, wq_c, wv_c], axis=1)  # [C, 384]
        in_maps.append({
            "xr": xb[b],
            "yr": yb[b],
            "wall": np.ascontiguousarray(wall).astype(bf16),
            "wp": np.ascontiguousarray(wp_c).astype(bf16),
        })
    return in_maps


def assemble_output(results, inputs):
    lb = np.asarray(inputs["lb"], np.float32)
    Wp = np.asarray(inputs["Wp"], np.float32)
    bp = np.asarray(inputs["bp"], np.float32)
    bias = (bp + lb @ Wp).astype(np.float32)
    parts = [np.asarray(results[i]["outT"], dtype=np.float32)
             for i in range(NCORES)]
    out = np.stack([
        parts[0] + parts[1] + parts[2] + parts[3],
        parts[4] + parts[5] + parts[6] + parts[7],
    ])
    out += bias[None, :, None]
    return out.astype(np.float32)


def kernel(x, y, Wq, Wkv, lw, lb, Wp, bp):
    global LAST_RUN
    from concourse.bass_utils import run_bass_kernel_spmd

    inputs = dict(x=x, y=y, Wq=Wq, Wkv=Wkv, lw=lw, lb=lb, Wp=Wp, bp=bp)
    nc = _get_program()
    in_maps = make_in_maps(inputs)
    LAST_RUN = run_bass_kernel_spmd(nc, in_maps, list(range(NCORES)))
    return assemble_output(LAST_RUN.results, inputs)



# revision 54
# speedup vs baseline: 1.0757x; 1.0254x over previous
"""Trainium2 Bass kernel for nn_CrossAttention (B=2, C=512, N=M=2048, H=8).

Sharding: batch*heads = 16 (b,h) pairs across 8 cores, 2 heads per core.
Cores 0-3 handle batch 0 (heads in pairs), cores 4-7 batch 1.

Per-core pipeline (bf16 compute, fp32 PSUM accumulation):
  kT[d,m] = Wk_cols.T @ y_b          (2 heads packed on partitions)
  qT[d,n] = (Wq_cols * SCALE).T @ x_b
  v2[m, 1+d | 1+d] = y_blk.T @ Wv'   (direct [m,d] layout, ones cols preset;
                                      Wv' has the depthwise conv folded in)
  S^T[m,n] = kT_h.T-slices @ qT_h    (row-packed K=64 pairs per head)
  P = exp(S^T) -> bf16               (ScalarE streaming [128,1024] blocks)
  acc[n, 1+d] += P_blk.T @ v2[m]     (flipped attnout: P is the stationary,
                                      65-wide free -> 65 cyc/matmul; col 0 of
                                      each group accumulates the denominator;
                                      one start/stop per PSUM bank since the
                                      start bit zeroes the whole 2KB bank)
  nrm[n, d] = acc * recip(den)       (DVE per-partition scalar multiply)
  attnT[c, n] = PE-transpose(nrm)    (bf16)
  outT_partial[cout, n] = Wp_rows.T @ attnT   (bf16 partials to HBM)

Host folds (1+lw) into Wv, bias' = bp + lb @ Wp (exact: softmax rows sum
to 1), sums the 4 per-batch partials, adds bias'.
"""

import os
import sys
import numpy as np
from contextlib import ExitStack

for _p in ("/root/.axon_site", "/root/.axon_site/_ro/trn_rl_repo",
           "/root/.axon_site/_ro/pypackages", "/opt/trn_rl_repo"):
    if os.path.isdir(_p) and _p not in sys.path:
        sys.path.append(_p)

B, C, N, M, H = 2, 512, 2048, 2048, 8
HD = C // H
SCALE = HD ** -0.5
NCORES = 8

_NC = None
LAST_RUN = None

# ---- custom DVE exp: exp(32*y) = poly4(y)^32 for y in [-0.4, 0.4] ----
# Offloads part of the softmax exp stream from the (bottleneck) Scalar
# engine to the Vector engine. Scores are pre-scaled by 1/32 (folded into
# Wq); ScalarE blocks use activation(..., scale=32) at no extra cost.
EXP_C0 = 0.5001447017887652
EXP_C1 = 0.16771833562855049
EXP_C2 = 0.04088734265090471

_EXP_OPS = {}


def _register_exp_ops():
    import numpy as np
    from concourse import dve_ops
    from concourse.dve_spec import Spec, Src0, C0, C1, C2, One, lower
    from concourse.dve_uop import DveOpSpec
    from concourse.dve_ops import DveOp, _SUB_OPCODE_FOR_NAME

    if _EXP_OPS:
        return _EXP_OPS

    _y = Src0
    _s = _y * _y
    poly_body = ((C2 * _s + C1 * _y + C0) * _s + _y) + One
    _q = Src0
    _q2 = _q * _q
    _q4 = _q2 * _q2
    _q8 = _q4 * _q4
    _q16 = _q8 * _q8
    sq32_body = _q16 * _q16

    def ref_poly(in0, in1, s0, s1, imm2):
        y = in0.astype(np.float32)
        s = (y * y).astype(np.float32)
        u = (np.float32(imm2) * s).astype(np.float32)
        v = (np.float32(s1) * y).astype(np.float32)
        w = ((u + v).astype(np.float32) + np.float32(s0)).astype(np.float32)
        z = ((w * s).astype(np.float32) + y).astype(np.float32)
        return (z + np.float32(1.0)).astype(np.float32)

    def ref_sq32(in0, in1, s0, s1, imm2):
        q = in0.astype(np.float32)
        for _ in range(5):
            q = (q * q).astype(np.float32)
        return q

    def make_op(name, body, ref):
        spec = Spec(body=body, reference=ref)
        tmp = DveOpSpec(name=name, opcode=0, uops=lower(spec, ver="v3"),
                        rd1_en=False)
        return DveOp(name, spec, subdim=False, uops_sha={"v3": tmp.sha("v3")})

    for name, body, ref in (("EXP_POLY_ANT", poly_body, ref_poly),
                            ("EXP_SQ32_ANT", sq32_body, ref_sq32)):
        if name not in _SUB_OPCODE_FOR_NAME:
            op = make_op(name, body, ref)
            row = max(_SUB_OPCODE_FOR_NAME.values()) + 1
            assert row < 0x20
            dve_ops.OPS.append(op)
            _SUB_OPCODE_FOR_NAME[name] = row
            dve_ops.CUSTOM_DVE_SPECS[name] = op.spec
        else:
            op = next(o for o in dve_ops.OPS if o.name == name)
        _EXP_OPS[name] = op
    return _EXP_OPS


DEFAULT_CFG = dict(warm=7, ppool=5, v2pro=4, m0split=True, defer=True,
                   lag=3, hold=3, tail="streams", tpc_act=True, dma2=True,
                   so_act=False, post0=2, m1early=False, divide=False,
                   divide_tail=False, v2shift=0, ktq=False, defer_m=14, v2tail=1, qoff=3,
                   postpack=1, dmaorder=True, v2pair=True, endlag=2,
                   post_steps=(3, 4, 7, 8, 10, 11, 12, 13, 14),
                   dvexp={(1, 1): 1024, (1, 6): 1024,
                          (2, 1): 1024, (2, 6): 1024,
                          (3, 1): 1024, (3, 6): 1024})


def _build_program(reps=1, cfg=None):
    cfg = dict(DEFAULT_CFG, **(cfg or {}))
    # the previous accumulator must be copied out (post0) only after its
    # last attnout has been emitted, which happens at step lag-1
    cfg["post0"] = max(cfg["post0"], cfg["lag"])
    exp_ops = _register_exp_ops()
    EXP_POLY = exp_ops["EXP_POLY_ANT"]
    EXP_SQ32 = exp_ops["EXP_SQ32_ANT"]
    dvexp = dict(cfg["dvexp"])
    from concourse import bacc
    import concourse.tile as tile
    import concourse.mybir as mybir
    from concourse.masks import make_identity

    F32 = mybir.dt.float32
    BF16 = mybir.dt.bfloat16
    EXP = mybir.ActivationFunctionType.Exp
    COPY = mybir.ActivationFunctionType.Copy
    MULT = mybir.AluOpType.mult
    DIV = mybir.AluOpType.divide

    nc = bacc.Bacc("TRN2", target_bir_lowering=False, debug=False,
                   num_devices=NCORES)

    xr = nc.dram_tensor("xr", [C, N], BF16, kind="ExternalInput").ap()
    yr = nc.dram_tensor("yr", [C, M], BF16, kind="ExternalInput").ap()
    # wall = [Wk' | Wq' | Wv'] concatenated so one DMA loads all three
    wall_d = nc.dram_tensor("wall", [C, 384], BF16, kind="ExternalInput").ap()
    wp_d = nc.dram_tensor("wp", [128, C], BF16, kind="ExternalInput").ap()
    outT = nc.dram_tensor("outT", [C, N], BF16, kind="ExternalOutput").ap()

    xr4 = xr.rearrange("(kc p) n -> p kc n", p=128)
    yr4 = yr.rearrange("(kc p) n -> p kc n", p=128)
    outT4 = outT.rearrange("(cc p) n -> p cc n", p=128)

    with tile.TileContext(nc) as tc, ExitStack() as ctx:
        sb = ctx.enter_context(tc.tile_pool(name="sb", bufs=1))
        ppool = ctx.enter_context(tc.tile_pool(name="ppool", bufs=cfg["ppool"]))
        qpool = ctx.enter_context(tc.tile_pool(name="qpool", bufs=2))
        npool = ctx.enter_context(tc.tile_pool(name="npool", bufs=2))
        spool = ctx.enter_context(tc.tile_pool(name="spool", bufs=2))
        # PSUM budget (8 banks): psA ring 3x[128,1024]f32 = 6 banks (scores,
        # proj/v2 staging, transposes, outproj transients); psB 2x1 bank
        # (attnout accumulators; the tail reuses them for outproj).
        psA = ctx.enter_context(tc.tile_pool(name="psA", bufs=3, space="PSUM"))
        psB = ctx.enter_context(tc.tile_pool(name="psB", bufs=2, space="PSUM"))

        # ---- PE warm-up with no DMA dependency: DVE-zeroed operand ----
        zwarm = sb.tile([128, 128], F32, tag="zwarm")
        nc.vector.memset(zwarm, 0.0)
        # warm the exp table while DMAs stream
        warm = sb.tile([1, 32], F32, tag="warm")
        nc.scalar.activation(warm, zwarm[0:1, 0:32], EXP)
        psw = psA.tile([128, 128], F32, tag="blk", name="psw")
        for _ in range(cfg["warm"]):
            nc.tensor.matmul(psw, zwarm, zwarm, start=True, stop=True)
        warm2 = sb.tile([128, 128], F32, tag="warm2")
        nc.vector.tensor_copy(warm2, psw)

        # ---- input DMAs, all on the sync-engine HWDGE queue; order is
        # the first-use order so the global DMA serialization helps the
        # prologue rather than hurting it ----
        wall_sb = sb.tile([128, 4, 384], BF16, tag="wall_sb")
        wp_sb = sb.tile([128, C], BF16, tag="wp_sb")
        y_sb = sb.tile([128, 4, M], BF16, tag="y_sb")
        x_sb = sb.tile([128, 4, N], BF16, tag="x_sb")
        wk_sb = wall_sb[:, :, 0:128]
        wq_sb = wall_sb[:, :, 128:256]
        wv_sb = wall_sb[:, :, 256:384]

        def load_j(dst, src, j):
            nc.sync.dma_start(out=dst[:, :, j * 512:(j + 1) * 512],
                              in_=src[:, :, j * 512:(j + 1) * 512])

        def load_half(dst, src, h):
            nc.sync.dma_start(out=dst[:, :, h * 256:(h + 1) * 256],
                              in_=src[:, :, h * 256:(h + 1) * 256])

        def load_cols(dst, src, c0, c1):
            nc.sync.dma_start(out=dst[:, :, c0:c1], in_=src[:, :, c0:c1])

        wall4 = wall_d.rearrange("(kc p) m -> p kc m", p=128)
        if cfg["dmaorder"]:
            # fine-grained first pieces: wk, then just enough y/x for the
            # first score piece, ordered by first use
            load_cols(wall_sb, wall4, 0, 128)     # wk
            load_cols(y_sb, yr4, 0, 128)          # kT m-block 0
            load_cols(wall_sb, wall4, 128, 256)   # wq
            load_half(x_sb, xr4, 0)               # qT n 0:256
            load_cols(y_sb, yr4, 128, 256)        # kT m-block 1
            load_half(x_sb, xr4, 1)               # qT n 256:512
            load_half(y_sb, yr4, 1)               # kT m-blocks 2,3
        else:
            nc.sync.dma_start(out=wall_sb[:, :, 0:256],
                              in_=wall4[:, :, 0:256])
            load_half(y_sb, yr4, 0)
            load_half(x_sb, xr4, 0)
            load_half(y_sb, yr4, 1)
            load_half(x_sb, xr4, 1)
        nc.sync.dma_start(out=wall_sb[:, :, 256:384], in_=wall4[:, :, 256:384])
        load_j(y_sb, yr4, 1)
        load_j(x_sb, xr4, 1)
        nc.sync.dma_start(out=wp_sb, in_=wp_d)
        load_j(y_sb, yr4, 2)
        load_j(x_sb, xr4, 2)
        load_j(y_sb, yr4, 3)
        load_j(x_sb, xr4, 3)

        # identity (bf16, for PE transposes) built on the idle GPSIMD
        identb = sb.tile([128, 128], BF16, tag="identb")
        make_identity(nc, identb)

        # v2[p, mb, col]: col 0 = ones (head a den), 1..64 = head a values,
        # col 65 = ones (head b den), 66..129 = head b values.
        v2 = sb.tile([128, 16, 130], BF16, tag="v2")
        nc.vector.memset(v2[:, :, 0:1], 1.0)
        nc.vector.memset(v2[:, :, 65:66], 1.0)

        kT = sb.tile([128, M], BF16, tag="kT")
        qT = sb.tile([128, N], BF16, tag="qT")

        hold = {}

        def proj_half(dst, w_sb, src, j, half, name):
            if half == 0:
                hold[name] = psA.tile([128, 512], F32, tag="blk", name=name)
            ps = hold[name]
            for kc in (0, 1) if half == 0 else (2, 3):
                nc.tensor.matmul(ps, w_sb[:, kc, :],
                                 src[:, kc, j * 512:(j + 1) * 512],
                                 start=(kc == 0), stop=(kc == 3))
            if half == 1:
                nc.vector.tensor_copy(dst[:, j * 512:(j + 1) * 512], ps)

        def v2_task(mb):
            ps = psA.tile([128, 128], F32, tag="blk", name=f"v2ps{mb}")
            ms = slice(mb * 128, (mb + 1) * 128)
            for kc in range(4):
                nc.tensor.matmul(ps, y_sb[:, kc, ms], wv_sb[:, kc, :],
                                 start=(kc == 0), stop=(kc == 3))
            # one strided copy for both head groups: [128,(2,64)] view
            nc.vector.tensor_copy(
                v2[:, mb, :].rearrange("p (g c) -> p g c", g=2)[:, :, 1:65],
                ps.rearrange("p (g c) -> p g c", g=2))

        def v2_task2(mb):
            # two m-blocks in ONE psA allocation (single bank): keeps the
            # score-ring alternation intact where v2 and proj fills would
            # otherwise put two allocations between consecutive score blocks
            ps = psA.tile([128, 256], F32, tag="blk", name=f"v2ps{mb}")
            for k, b in enumerate((mb, mb + 1)):
                msl = slice(b * 128, (b + 1) * 128)
                for kc in range(4):
                    nc.tensor.matmul(ps[:, k * 128:(k + 1) * 128],
                                     y_sb[:, kc, msl], wv_sb[:, kc, :],
                                     start=(b == mb and kc == 0),
                                     stop=(b == mb + 1 and kc == 3))
            for k, b in enumerate((mb, mb + 1)):
                nc.vector.tensor_copy(
                    v2[:, b, :].rearrange("p (g c) -> p g c", g=2)[:, :, 1:65],
                    ps[:, k * 128:(k + 1) * 128].rearrange(
                        "p (g c) -> p g c", g=2))

        def proj256(dst, w_sb, src, h, name):
            # 256-wide projection so work starts at half-DMA arrival
            ps = psA.tile([128, 256], F32, tag="blk", name=name)
            sl = slice(h * 256, (h + 1) * 256)
            for kc in range(4):
                nc.tensor.matmul(ps, w_sb[:, kc, :], src[:, kc, sl],
                                 start=(kc == 0), stop=(kc == 3))
            nc.vector.tensor_copy(dst[:, sl], ps)

        # ---- prologue: only what gates scores(chunk0, m=0..3); the qT
        # copies go ahead of the v2 copies on the DVE queue since the
        # first exp gates on qT while v2 is only needed two steps in ----
        P00 = None
        if cfg["m0split"]:
            P00 = ppool.tile([128, 1024], BF16, tag="p", name="p0_0")

        def m0_piece(h):
            # tile_position'd matmuls may not share a PSUM bank: head
            # pieces go to separate banks, exp reads them strided.
            nh = slice(h * 256, (h + 1) * 256)
            blk = psA.tile([128, 2, 512], F32, tag="blk", name=f"blk0_0{h}")
            nc.tensor.matmul(blk[:, 0, 0:256], kT[0:64, 0:128],
                             qT[0:64, nh], start=True, stop=True,
                             tile_position=(0, 0))
            nc.tensor.matmul(blk[:, 1, 0:256], kT[64:128, 0:128],
                             qT[64:128, nh], start=True, stop=True,
                             tile_position=(64, 0))
            nc.scalar.activation(P00[:, h * 512:(h + 1) * 512],
                                 blk[:, :, 0:256], EXP, scale=32.0)

        proj256(kT, wk_sb, y_sb, 0, "psk0a")
        proj256(qT, wq_sb, x_sb, 0, "psq0a")
        if cfg["m0split"]:
            m0_piece(0)
        proj256(kT, wk_sb, y_sb, 1, "psk0b")
        proj256(qT, wq_sb, x_sb, 1, "psq0b")
        if cfg["m0split"]:
            m0_piece(1)
        P01 = None
        if cfg["m1early"]:
            # m=1 scores+exp also ahead of the v2 prologue block so the
            # exp stream has no gap while v2 builds
            blk01 = psA.tile([128, 1024], F32, tag="blk", name="blk0_1")
            nc.tensor.matmul(blk01[:, 0:512], kT[0:64, 128:256],
                             qT[0:64, 0:512], start=True, stop=True,
                             tile_position=(0, 0))
            nc.tensor.matmul(blk01[:, 512:1024], kT[64:128, 128:256],
                             qT[64:128, 0:512], start=True, stop=True,
                             tile_position=(64, 0))
            P01 = ppool.tile([128, 1024], BF16, tag="p", name="p0_1")
            nc.scalar.activation(P01, blk01, EXP, scale=32.0)
        # early v2 blocks ride the PE while later DMAs are in flight
        for _mb in range(cfg["v2pro"]):
            v2_task(_mb)

        # weave tasks: late projections + v2 blocks. Emission order defines
        # data dependencies (a consumer emitted before its producer reads
        # stale SBUF), so each task must be emitted strictly before its
        # first consumer: kT j before scores(0, 4j), v2(mb) before the
        # attnout(mb) emission (mb+2), qT j before scores(j, 0).
        def P_(dst, w, src, j, half, name):
            return lambda: proj_half(dst, w, src, j, half, name)

        def proj_q(dst, w_sb, src, j, kc, name):
            if kc == 0:
                hold[name] = psA.tile([128, 512], F32, tag="blk", name=name)
            ps = hold[name]
            nc.tensor.matmul(ps, w_sb[:, kc, :],
                             src[:, kc, j * 512:(j + 1) * 512],
                             start=(kc == 0), stop=(kc == 3))
            if kc == 3:
                nc.vector.tensor_copy(dst[:, j * 512:(j + 1) * 512], ps)

        def Q_(dst, w, src, j, kc, name):
            return lambda: proj_q(dst, w, src, j, kc, name)

        _v2rem = list(range(cfg["v2pro"], 4))
        _v2h = (len(_v2rem) + 1) // 2
        _vs = cfg["v2shift"]
        fills = {
            (0, 0): [(lambda mb=mb: v2_task(mb)) for mb in _v2rem[:_v2h]],
            (0, 1): [(lambda mb=mb: v2_task(mb)) for mb in _v2rem[_v2h:]],
            (0, 2): [P_(kT, wk_sb, y_sb, 1, 0, "psk1")],
            (0, 3): [P_(kT, wk_sb, y_sb, 1, 1, "psk1")],
        }
        _qo = cfg["qoff"]
        fills[(1, _qo)] = [P_(qT, wq_sb, x_sb, 2, 0, "psq2")]
        fills[(1, _qo + 1)] = [P_(qT, wq_sb, x_sb, 2, 1, "psq2")]
        fills[(2, _qo)] = [P_(qT, wq_sb, x_sb, 3, 0, "psq3")]
        fills[(2, _qo + 1)] = [P_(qT, wq_sb, x_sb, 3, 1, "psq3")]
        if cfg["ktq"]:
            # quarter-granular late projections: one matmul per weave slot
            for i, (j, kc) in enumerate([(2, 0), (2, 1), (2, 2), (2, 3),
                                         (3, 0), (3, 1), (3, 2), (3, 3)]):
                fills.setdefault((0, 4 + i), []).append(
                    Q_(kT, wk_sb, y_sb, j, kc, f"psk{j}"))
            for kc in range(4):
                fills.setdefault((0, 12 + kc), []).append(
                    Q_(qT, wq_sb, x_sb, 1, kc, "psq1"))
        else:
            fills[(0, 6)] = [P_(kT, wk_sb, y_sb, 2, 0, "psk2")]
            fills[(0, 7)] = [P_(kT, wk_sb, y_sb, 2, 1, "psk2")]
            fills[(0, 10)] = [P_(kT, wk_sb, y_sb, 3, 0, "psk3")]
            fills[(0, 11)] = [P_(kT, wk_sb, y_sb, 3, 1, "psk3")]
            fills[(0, 13)] = [P_(qT, wq_sb, x_sb, 1, 0, "psq1")]
            fills[(0, 14)] = [P_(qT, wq_sb, x_sb, 1, 1, "psq1")]
        if cfg["v2pair"]:
            # pairs on proj-free steps; each pair's deadline is the attnout
            # pop of its first block (step mb+lag)
            for mb, step in ((4, 4), (6, 5), (8, 8), (10, 9), (12, 12),
                             (14, 15)):
                fills.setdefault((0, step), []).append(
                    lambda mb=mb: v2_task2(mb))
        else:
            for mb in range(4, 16):
                # v2(mb) must be emitted before attnout(mb) pops at step
                # mb+lag (which may fall in chunk 1); shift within that
                # window to smooth the chunk-0 PE load
                step = mb + _vs + (cfg["v2tail"] if mb >= 14 else 0)
                key = (0, step) if step <= 15 else (1, step - 16)
                fills.setdefault(key, []).append(lambda mb=mb: v2_task(mb))

        # ---- main attention loop ----
        pending = []         # (P, acc_a, acc_b, m) awaiting attnout
        post = []            # deferred post-processing closures

        def emit_attnout(P, acc_a, acc_b, m):
            # PSUM zero-region = one full 2KB bank: exactly one start
            # (m=0,s=0) and one stop (m=15,s=3) per accumulator tile.
            for s in range(4):
                if isinstance(P, tuple):  # split first m-step: [a256|b256]x2
                    Pt = P[1]
                    h, i = s // 2, s % 2
                    pa = Pt[:, h * 512 + i * 128:h * 512 + (i + 1) * 128]
                    pb = Pt[:, h * 512 + 256 + i * 128:
                            h * 512 + 256 + (i + 1) * 128]
                else:
                    pa = P[:, s * 128:(s + 1) * 128]
                    pb = P[:, 512 + s * 128:512 + (s + 1) * 128]
                nc.tensor.matmul(acc_a[:, s, :], pa, v2[:, m, 0:65],
                                 start=(m == 0 and s == 0),
                                 stop=(m == 15 and s == 3))
                nc.tensor.matmul(acc_b[:, s, :], pb, v2[:, m, 65:130],
                                 start=(m == 0 and s == 0),
                                 stop=(m == 15 and s == 3))

        def make_post(c, acc_a, acc_b):
            st = {}

            def grab(aps, dst_tag, out_name):
                # one fast PSUM->SBUF copy releases the accumulator bank
                t = spool.tile([128, 4, 65], F32, tag=dst_tag, name=out_name)
                nc.vector.tensor_copy(t, aps)
                return t

            def grab_a():
                st["ca"] = grab(acc_a, "ca", f"ca{c}")
                st["cb"] = grab(acc_b, "cb", f"cb{c}")

            def recips():
                st["attnT"] = npool.tile([128, 512], BF16, tag="attnT",
                                         name=f"attnT{c}")
                st["ra"] = spool.tile([128, 4], F32, tag="ra", name=f"ra{c}")
                st["rb"] = spool.tile([128, 4], F32, tag="rb", name=f"rb{c}")
                nc.vector.reciprocal(st["ra"], st["ca"][:, :, 0:1])
                nc.vector.reciprocal(st["rb"], st["cb"][:, :, 0:1])

            def norm():
                # both heads' normalize in two broadcast multiplies
                st["trin"] = spool.tile([128, 4, 128], BF16, tag="trin",
                                        name=f"trin{c}")
                ra_b = st["ra"][:, :, None].broadcast_to([128, 4, 64])
                rb_b = st["rb"][:, :, None].broadcast_to([128, 4, 64])
                nc.vector.tensor_tensor(out=st["trin"][:, :, 0:64],
                                        in0=st["ca"][:, :, 1:65],
                                        in1=ra_b, op=MULT)
                nc.vector.tensor_tensor(out=st["trin"][:, :, 64:128],
                                        in0=st["cb"][:, :, 1:65],
                                        in1=rb_b, op=MULT)

            def sub(s):
                # paired transposes share one PSUM bank (start zeroes it,
                # second adds into the disjoint half) and one wide copy
                tp = psA.tile([128, 2, 128], BF16, tag="blk",
                              name=f"tp{c}_{s}")
                for k in (0, 1):
                    nc.tensor.matmul(tp[:, k, :], st["trin"][:, s + k, :],
                                     identb, is_transpose=True,
                                     start=(k == 0), stop=(k == 1))
                nc.vector.tensor_copy(
                    st["attnT"][:, s * 128:(s + 2) * 128]
                    .rearrange("p (a b) -> p a b", a=2), tp)

            def outproj(cc):
                po = psA.tile([128, 512], F32, tag="blk", name=f"po{c}_{cc}")
                nc.tensor.matmul(po, wp_sb[:, cc * 128:(cc + 1) * 128],
                                 st["attnT"], start=True, stop=True)
                if cc == 0:
                    st["so"] = npool.tile([128, 4, 512], BF16, tag="so",
                                          name=f"so{c}")
                if cfg["so_act"] and cc % 2 == 1:
                    nc.scalar.copy(st["so"][:, cc, :], po)
                else:
                    nc.vector.tensor_copy(st["so"][:, cc, :], po)
                if cc == 3:
                    nc.sync.dma_start(
                        out=outT4[:, :, c * 512:(c + 1) * 512],
                        in_=st["so"])

            tasks = ([[grab_a], [recips], [norm]]
                     + [[lambda s=s: sub(s)] for s in (0, 2)]
                     + [[lambda cc=cc: outproj(cc)] for cc in range(4)])
            return tasks

        for c in range(4):
            ns = slice(c * 512, (c + 1) * 512)
            acc_a = psB.tile([128, 4, 65], F32, tag="acc", name=f"acca{c}")
            acc_b = psB.tile([128, 4, 65], F32, tag="acc", name=f"accb{c}")
            sched = None
            if cfg["post_steps"] is not None and post:
                sched = {}
                for i, grp in enumerate(post):
                    sched.setdefault(cfg["post_steps"][i], []).append(grp)
                post = []
            for m in range(16):
                ms = slice(m * 128, (m + 1) * 128)
                if cfg["m0split"] and c == 0 and m == 0:
                    # scores+exp for m0 were already emitted in the
                    # prologue (split into two x-half-gated pieces with
                    # layout [a n256 | b n256] per half)
                    P = ("split", P00)
                elif cfg["m1early"] and c == 0 and m == 1:
                    P = P01
                else:
                    blk = psA.tile([128, 1024], F32, tag="blk",
                                   name=f"blk{c}_{m}")
                    nc.tensor.matmul(blk[:, 0:512], kT[0:64, ms],
                                     qT[0:64, ns],
                                     start=True, stop=True,
                                     tile_position=(0, 0))
                    nc.tensor.matmul(blk[:, 512:1024], kT[64:128, ms],
                                     qT[64:128, ns],
                                     start=True, stop=True,
                                     tile_position=(64, 0))
                    P = ppool.tile([128, 1024], BF16, tag="p",
                                   name=f"p{c}_{m}")
                    dn = dvexp.get((c, m), 0)
                    if dn:
                        # split the exp: ScalarE takes cols [0:1024-dn],
                        # DVE (poly + 5 squarings) takes the tail [1024-dn:]
                        x0 = 1024 - dn
                        if x0:
                            nc.scalar.activation(P[:, 0:x0], blk[:, 0:x0],
                                                 EXP, scale=32.0)
                        q = qpool.tile([128, dn], F32, tag="q",
                                       name=f"q{c}_{m}")
                        nc.vector._custom_dve(EXP_POLY, out=q,
                                              in0=blk[:, x0:1024],
                                              s0=EXP_C0, s1=EXP_C1,
                                              imm2=EXP_C2)
                        nc.vector._custom_dve(EXP_SQ32, out=P[:, x0:1024],
                                              in0=q)
                    else:
                        nc.scalar.activation(P, blk, EXP, scale=32.0)
                if sched is not None:
                    for grp in sched.pop(m, ()):
                        for task in grp:
                            task()
                elif post and m >= cfg["post0"]:
                    for task in post.pop(0):
                        task()
                for task in fills.pop((c, m), ()):
                    task()
                pending.append((P, acc_a, acc_b, m))
                # lag 3 mid-chunk; at m=14/15 drain the backlog down to 1
                # (PE has slack there) so chunk boundaries don't pile
                # attnouts on top of the next chunk's first scores; a new
                # chunk's first attnout (which waits for the previous
                # accumulator bank to be copied out by grab_a) is held until
                # m=3; the last chunk defers its final steps entirely.
                lag_now = cfg["lag"] if m < 14 else (
                    cfg["endlag"] if m == 14 else cfg["endlag"] - 1)
                while len(pending) > lag_now and not (
                        c > 0 and m < cfg["hold"] and
                        pending[0][3] == 0) and not (
                        cfg["defer"] and c == 3 and m >= cfg["defer_m"]):
                    emit_attnout(*pending.pop(0))
            post = make_post(c, acc_a, acc_b)

        # ---- drain + tail (chunk 3 post-processing, pipelined) ----
        # Read the accumulators straight from PSUM (no ring pressure at
        # the end), split normalize/copy work across DVE and the now-idle
        # ScalarE (Copy shares the exp activation table, no reload), and
        # run the output projection per n-sub-block so copies and DMAs
        # start as early as possible. Two po tiles reuse the accumulator
        # banks freed at the start of the tail.
        while pending:
            emit_attnout(*pending.pop(0))
        c = 3
        ra = spool.tile([128, 4], F32, tag="ra", name="ra3")
        rb = spool.tile([128, 4], F32, tag="rb", name="rb3")
        nc.vector.reciprocal(ra, acc_a[:, :, 0:1])
        nc.vector.reciprocal(rb, acc_b[:, :, 0:1])
        # normalize both heads with two broadcast multiplies (DVE)
        trin3 = spool.tile([128, 4, 128], BF16, tag="trin", name="trin3")
        nc.vector.tensor_tensor(out=trin3[:, :, 0:64], in0=acc_a[:, :, 1:65],
                                in1=ra[:, :, None].broadcast_to([128, 4, 64]),
                                op=MULT)
        nc.vector.tensor_tensor(out=trin3[:, :, 64:128],
                                in0=acc_b[:, :, 1:65],
                                in1=rb[:, :, None].broadcast_to([128, 4, 64]),
                                op=MULT)
        po = [psB.tile([128, 512], F32, tag="acc", name="po3_0"),
              psB.tile([128, 512], F32, tag="acc", name="po3_1"),
              psA.tile([128, 512], F32, tag="blk", name="po3_2"),
              psA.tile([128, 512], F32, tag="blk", name="po3_3")]
        # paired transposes -> two half-attnT tiles with one writer each so
        # the PSUM->SBUF copies run concurrently on DVE and ScalarE
        attnTh = [npool.tile([128, 2, 128], BF16, tag=f"attnT3_{h}",
                             name=f"attnT3_{h}") for h in (0, 1)]
        for h in (0, 1):
            tp = psA.tile([128, 2, 128], BF16, tag="blk", name=f"tp3_{h}")
            for k in (0, 1):
                nc.tensor.matmul(tp[:, k, :], trin3[:, 2 * h + k, :],
                                 identb, is_transpose=True,
                                 start=(k == 0), stop=(k == 1))
            if h == 0:
                nc.vector.tensor_copy(attnTh[h], tp)
            else:
                nc.scalar.copy(attnTh[h], tp)
        for s in range(4):
            for cc in range(4):
                nc.tensor.matmul(po[cc][:, s * 128:(s + 1) * 128],
                                 wp_sb[:, cc * 128:(cc + 1) * 128],
                                 attnTh[s // 2][:, s % 2, :],
                                 start=(s == 0), stop=(s == 3))
        so3 = npool.tile([128, 4, 512], BF16, tag="so", name="so3")
        nc.vector.tensor_copy(so3[:, 0, :], po[0])
        nc.scalar.copy(so3[:, 1, :], po[1])
        nc.sync.dma_start(out=outT4[:, 0:2, c * 512:(c + 1) * 512],
                          in_=so3[:, 0:2, :])
        nc.vector.tensor_copy(so3[:, 2, :], po[2])
        nc.scalar.copy(so3[:, 3, :], po[3])
        nc.sync.dma_start(out=outT4[:, 2:4, c * 512:(c + 1) * 512],
                          in_=so3[:, 2:4, :])

    nc.compile()
    return nc


def _get_program():
    global _NC
    if _NC is None:
        _NC = _build_program()
    return _NC


def make_in_maps(inputs):
    import ml_dtypes
    bf16 = ml_dtypes.bfloat16

    x = np.asarray(inputs["x"], np.float32)
    y = np.asarray(inputs["y"], np.float32)
    Wq = np.asarray(inputs["Wq"], np.float32)
    Wkv = np.asarray(inputs["Wkv"], np.float32)
    lw = np.asarray(inputs["lw"], np.float32)
    Wp = np.asarray(inputs["Wp"], np.float32)

    d = np.arange(HD)
    xb = [np.ascontiguousarray(x[b]).astype(bf16) for b in range(B)]
    yb = [np.ascontiguousarray(y[b]).astype(bf16) for b in range(B)]
    in_maps = []
    for core in range(NCORES):
        b = core // 4
        h0 = (core % 4) * 2
        ch = np.concatenate([h * HD + d for h in (h0, h0 + 1)])  # channels
        colsK = np.concatenate([h * 2 * HD + 2 * d for h in (h0, h0 + 1)])
        wq_c = Wq[:, ch] * np.float32(SCALE / 32.0)
        wk_c = Wkv[:, colsK]
        wv_c = Wkv[:, colsK + 1] * (1.0 + lw[ch])[None, :]
        wp_c = Wp[ch, :]
        wall = np.concatenate([wk_c, wq_c, wv_c], axis=1)  # [C, 384]
        in_maps.append({
            "xr": xb[b],
            "yr": yb[b],
            "wall": np.ascontiguousarray(wall).astype(bf16),
            "wp": np.ascontiguousarray(wp_c).astype(bf16),
        })
    return in_maps


def assemble_output(results, inputs):
    lb = np.asarray(inputs["lb"], np.float32)
    Wp = np.asarray(inputs["Wp"], np.float32)
    bp = np.asarray(inputs["bp"], np.float32)
    bias = (bp + lb @ Wp).astype(np.float32)
    parts = [np.asarray(results[i]["outT"], dtype=np.float32)
             for i in range(NCORES)]
    out = np.stack([
        parts[0] + parts[1] + parts[2] + parts[3],
        parts[4] + parts[5] + parts[6] + parts[7],
    ])
    out += bias[None, :, None]
    return out.astype(np.float32)


def kernel(x, y, Wq, Wkv, lw, lb, Wp, bp):
    global LAST_RUN
    from concourse.bass_utils import run_bass_kernel_spmd

    inputs = dict(x=x, y=y, Wq=Wq, Wkv=Wkv, lw=lw, lb=lb, Wp=Wp, bp=bp)
    nc = _get_program()
    in_maps = make_in_maps(inputs)
    LAST_RUN = run_bass_kernel_spmd(nc, in_maps, list(range(NCORES)))
    return assemble_output(LAST_RUN.results, inputs)



# revision 64
# speedup vs baseline: 1.0800x; 1.0040x over previous
"""Trainium2 Bass kernel for nn_CrossAttention (B=2, C=512, N=M=2048, H=8).

Sharding: batch*heads = 16 (b,h) pairs across 8 cores, 2 heads per core.
Cores 0-3 handle batch 0 (heads in pairs), cores 4-7 batch 1.

Per-core pipeline (bf16 compute, fp32 PSUM accumulation):
  kT[d,m] = Wk_cols.T @ y_b          (2 heads packed on partitions)
  qT[d,n] = (Wq_cols * SCALE/32).T @ x_b   (scores pre-scaled by 1/32)
  v2[m, 1+d | 1+d] = y_blk.T @ Wv'   (direct [m,d] layout, ones cols preset;
                                      Wv' has the depthwise conv folded in;
                                      paired m-blocks per PSUM allocation)
  S^T[m,n] = kT_h.T-slices @ qT_h    (row-packed K=64 pairs per head)
  P = exp(32*S^T) -> bf16            (ScalarE activation with scale=32 for
                                      58 blocks; 6 blocks go to the Vector
                                      engine via two custom-DVE ops:
                                      poly4(y) then ^32 by 5 squarings --
                                      see _register_exp_ops. Splitting one
                                      block across both engines does not
                                      work: cross-engine readers of one
                                      tile are serialized by Tile.)
  acc[n, 1+d] += P_blk.T @ v2[m]     (flipped attnout: P is the stationary,
                                      65-wide free -> 65 cyc/matmul; col 0 of
                                      each group accumulates the denominator;
                                      one start/stop per PSUM bank since the
                                      start bit zeroes the whole 2KB bank;
                                      backlog drained at m=14/15 so chunk
                                      boundaries don't pile attnouts onto
                                      the next chunk's first scores)
  nrm[n, d] = acc / den              (one broadcast DVE op per head)
  attnT[c, n] = PE-transpose(nrm)    (bf16, paired into one PSUM bank via
                                      start/stop accumulate + one wide copy)
  outT_partial[cout, n] = Wp_rows.T @ attnT   (bf16 partials to HBM)

Post-processing of chunk c runs during chunk c+1 on an explicit step
schedule (post_steps) that keeps at most one non-score PSUM allocation
between consecutive score blocks -- two allocations in between would put
consecutive score blocks on the same psA ring slot and serialize the
exp stream (the psA ring has 3 slots: writer, exp reader, transient).

Host folds (1+lw) into Wv, bias' = bp + lb @ Wp (exact: softmax rows sum
to 1), sums the 4 per-batch partials, adds bias'.
"""

import os
import sys
import numpy as np
from contextlib import ExitStack

for _p in ("/root/.axon_site", "/root/.axon_site/_ro/trn_rl_repo",
           "/root/.axon_site/_ro/pypackages", "/opt/trn_rl_repo"):
    if os.path.isdir(_p) and _p not in sys.path:
        sys.path.append(_p)

B, C, N, M, H = 2, 512, 2048, 2048, 8
HD = C // H
SCALE = HD ** -0.5
NCORES = 8

_NC = None
LAST_RUN = None

# ---- custom DVE exp: exp(32*y) = poly4(y)^32 for y in [-0.4, 0.4] ----
# Offloads part of the softmax exp stream from the (bottleneck) Scalar
# engine to the Vector engine. Scores are pre-scaled by 1/32 (folded into
# Wq); ScalarE blocks use activation(..., scale=32) at no extra cost.
EXP_C0 = 0.5001447017887652
EXP_C1 = 0.16771833562855049
EXP_C2 = 0.04088734265090471

_EXP_OPS = {}


def _register_exp_ops():
    import numpy as np
    from concourse import dve_ops
    from concourse.dve_spec import Spec, Src0, C0, C1, C2, One, lower
    from concourse.dve_uop import DveOpSpec
    from concourse.dve_ops import DveOp, _SUB_OPCODE_FOR_NAME

    if _EXP_OPS:
        return _EXP_OPS

    _y = Src0
    _s = _y * _y
    poly_body = ((C2 * _s + C1 * _y + C0) * _s + _y) + One
    _q = Src0
    _q2 = _q * _q
    _q4 = _q2 * _q2
    _q8 = _q4 * _q4
    _q16 = _q8 * _q8
    sq32_body = _q16 * _q16

    def ref_poly(in0, in1, s0, s1, imm2):
        y = in0.astype(np.float32)
        s = (y * y).astype(np.float32)
        u = (np.float32(imm2) * s).astype(np.float32)
        v = (np.float32(s1) * y).astype(np.float32)
        w = ((u + v).astype(np.float32) + np.float32(s0)).astype(np.float32)
        z = ((w * s).astype(np.float32) + y).astype(np.float32)
        return (z + np.float32(1.0)).astype(np.float32)

    def ref_sq32(in0, in1, s0, s1, imm2):
        q = in0.astype(np.float32)
        for _ in range(5):
            q = (q * q).astype(np.float32)
        return q

    def make_op(name, body, ref):
        spec = Spec(body=body, reference=ref)
        tmp = DveOpSpec(name=name, opcode=0, uops=lower(spec, ver="v3"),
                        rd1_en=False)
        return DveOp(name, spec, subdim=False, uops_sha={"v3": tmp.sha("v3")})

    for name, body, ref in (("EXP_POLY_ANT", poly_body, ref_poly),
                            ("EXP_SQ32_ANT", sq32_body, ref_sq32)):
        if name not in _SUB_OPCODE_FOR_NAME:
            op = make_op(name, body, ref)
            row = max(_SUB_OPCODE_FOR_NAME.values()) + 1
            assert row < 0x20
            dve_ops.OPS.append(op)
            _SUB_OPCODE_FOR_NAME[name] = row
            dve_ops.CUSTOM_DVE_SPECS[name] = op.spec
        else:
            op = next(o for o in dve_ops.OPS if o.name == name)
        _EXP_OPS[name] = op
    return _EXP_OPS


DEFAULT_CFG = dict(warm=7, ppool=5, v2pro=4, m0split=True, defer=True,
                   lag=3, hold=3, tpc_act=True,
                   so_act=False, post0=2, m1early=False, divide=False,
                   divide_tail=False, v2shift=0, ktq=False, defer_m=14, v2tail=1, qoff=3,
                   dmaorder=False, v2pair=True, endlag=3,
                   post_steps=(3, 4, 7, 8, 10, 11, 12, 13, 14),
                   dvexp={(1, 1): 1024, (1, 6): 1024,
                          (2, 1): 1024, (2, 6): 1024,
                          (3, 1): 1024, (3, 6): 1024})


def _build_program(reps=1, cfg=None):
    cfg = dict(DEFAULT_CFG, **(cfg or {}))
    # the previous accumulator must be copied out (post0) only after its
    # last attnout has been emitted, which happens at step lag-1
    cfg["post0"] = max(cfg["post0"], cfg["lag"])
    exp_ops = _register_exp_ops()
    EXP_POLY = exp_ops["EXP_POLY_ANT"]
    EXP_SQ32 = exp_ops["EXP_SQ32_ANT"]
    dvexp = dict(cfg["dvexp"])
    from concourse import bacc
    import concourse.tile as tile
    import concourse.mybir as mybir
    from concourse.masks import make_identity

    F32 = mybir.dt.float32
    BF16 = mybir.dt.bfloat16
    EXP = mybir.ActivationFunctionType.Exp
    COPY = mybir.ActivationFunctionType.Copy
    MULT = mybir.AluOpType.mult
    DIV = mybir.AluOpType.divide

    nc = bacc.Bacc("TRN2", target_bir_lowering=False, debug=False,
                   num_devices=NCORES)

    xr = nc.dram_tensor("xr", [C, N], BF16, kind="ExternalInput").ap()
    yr = nc.dram_tensor("yr", [C, M], BF16, kind="ExternalInput").ap()
    # wall = [Wk' | Wq' | Wv'] concatenated so one DMA loads all three
    wall_d = nc.dram_tensor("wall", [C, 384], BF16, kind="ExternalInput").ap()
    wp_d = nc.dram_tensor("wp", [128, C], BF16, kind="ExternalInput").ap()
    outT = nc.dram_tensor("outT", [C, N], BF16, kind="ExternalOutput").ap()

    xr4 = xr.rearrange("(kc p) n -> p kc n", p=128)
    yr4 = yr.rearrange("(kc p) n -> p kc n", p=128)
    outT4 = outT.rearrange("(cc p) n -> p cc n", p=128)

    with tile.TileContext(nc) as tc, ExitStack() as ctx:
        sb = ctx.enter_context(tc.tile_pool(name="sb", bufs=1))
        ppool = ctx.enter_context(tc.tile_pool(name="ppool", bufs=cfg["ppool"]))
        qpool = ctx.enter_context(tc.tile_pool(name="qpool", bufs=2))
        npool = ctx.enter_context(tc.tile_pool(name="npool", bufs=2))
        spool = ctx.enter_context(tc.tile_pool(name="spool", bufs=2))
        # PSUM budget (8 banks): psA ring 3x[128,1024]f32 = 6 banks (scores,
        # proj/v2 staging, transposes, outproj transients); psB 2x1 bank
        # (attnout accumulators; the tail reuses them for outproj).
        psA = ctx.enter_context(tc.tile_pool(name="psA", bufs=3, space="PSUM"))
        psB = ctx.enter_context(tc.tile_pool(name="psB", bufs=2, space="PSUM"))

        # ---- PE warm-up with no DMA dependency: DVE-zeroed operand ----
        zwarm = sb.tile([128, 128], F32, tag="zwarm")
        nc.vector.memset(zwarm, 0.0)
        # warm the exp table while DMAs stream
        warm = sb.tile([1, 32], F32, tag="warm")
        nc.scalar.activation(warm, zwarm[0:1, 0:32], EXP)
        psw = psA.tile([128, 128], F32, tag="blk", name="psw")
        for _ in range(cfg["warm"]):
            nc.tensor.matmul(psw, zwarm, zwarm, start=True, stop=True)
        warm2 = sb.tile([128, 128], F32, tag="warm2")
        nc.vector.tensor_copy(warm2, psw)

        # ---- input DMAs, all on the sync-engine HWDGE queue; order is
        # the first-use order so the global DMA serialization helps the
        # prologue rather than hurting it ----
        wall_sb = sb.tile([128, 4, 384], BF16, tag="wall_sb")
        wp_sb = sb.tile([128, C], BF16, tag="wp_sb")
        y_sb = sb.tile([128, 4, M], BF16, tag="y_sb")
        x_sb = sb.tile([128, 4, N], BF16, tag="x_sb")
        wk_sb = wall_sb[:, :, 0:128]
        wq_sb = wall_sb[:, :, 128:256]
        wv_sb = wall_sb[:, :, 256:384]

        def load_j(dst, src, j):
            nc.sync.dma_start(out=dst[:, :, j * 512:(j + 1) * 512],
                              in_=src[:, :, j * 512:(j + 1) * 512])

        def load_half(dst, src, h):
            nc.sync.dma_start(out=dst[:, :, h * 256:(h + 1) * 256],
                              in_=src[:, :, h * 256:(h + 1) * 256])

        wall4 = wall_d.rearrange("(kc p) m -> p kc m", p=128)
        nc.sync.dma_start(out=wall_sb[:, :, 0:256], in_=wall4[:, :, 0:256])
        load_half(y_sb, yr4, 0)
        load_half(x_sb, xr4, 0)
        if cfg["dmaorder"]:
            load_half(x_sb, xr4, 1)
            load_half(y_sb, yr4, 1)
        else:
            load_half(y_sb, yr4, 1)
            load_half(x_sb, xr4, 1)
        nc.sync.dma_start(out=wall_sb[:, :, 256:384], in_=wall4[:, :, 256:384])
        load_j(y_sb, yr4, 1)
        load_j(x_sb, xr4, 1)
        nc.sync.dma_start(out=wp_sb, in_=wp_d)
        load_j(y_sb, yr4, 2)
        load_j(x_sb, xr4, 2)
        load_j(y_sb, yr4, 3)
        load_j(x_sb, xr4, 3)

        # identity (bf16, for PE transposes) built on the idle GPSIMD
        identb = sb.tile([128, 128], BF16, tag="identb")
        make_identity(nc, identb)

        # v2[p, mb, col]: col 0 = ones (head a den), 1..64 = head a values,
        # col 65 = ones (head b den), 66..129 = head b values.
        v2 = sb.tile([128, 16, 130], BF16, tag="v2")
        nc.vector.memset(v2[:, :, 0:1], 1.0)
        nc.vector.memset(v2[:, :, 65:66], 1.0)

        kT = sb.tile([128, M], BF16, tag="kT")
        qT = sb.tile([128, N], BF16, tag="qT")

        hold = {}

        def proj_half(dst, w_sb, src, j, half, name):
            if half == 0:
                hold[name] = psA.tile([128, 512], F32, tag="blk", name=name)
            ps = hold[name]
            for kc in (0, 1) if half == 0 else (2, 3):
                nc.tensor.matmul(ps, w_sb[:, kc, :],
                                 src[:, kc, j * 512:(j + 1) * 512],
                                 start=(kc == 0), stop=(kc == 3))
            if half == 1:
                nc.vector.tensor_copy(dst[:, j * 512:(j + 1) * 512], ps)

        def v2_task(mb):
            ps = psA.tile([128, 128], F32, tag="blk", name=f"v2ps{mb}")
            ms = slice(mb * 128, (mb + 1) * 128)
            for kc in range(4):
                nc.tensor.matmul(ps, y_sb[:, kc, ms], wv_sb[:, kc, :],
                                 start=(kc == 0), stop=(kc == 3))
            # one strided copy for both head groups: [128,(2,64)] view
            nc.vector.tensor_copy(
                v2[:, mb, :].rearrange("p (g c) -> p g c", g=2)[:, :, 1:65],
                ps.rearrange("p (g c) -> p g c", g=2))

        def v2_task2(mb):
            # two m-blocks in ONE psA allocation (single bank): keeps the
            # score-ring alternation intact where v2 and proj fills would
            # otherwise put two allocations between consecutive score blocks
            ps = psA.tile([128, 256], F32, tag="blk", name=f"v2ps{mb}")
            for k, b in enumerate((mb, mb + 1)):
                msl = slice(b * 128, (b + 1) * 128)
                for kc in range(4):
                    nc.tensor.matmul(ps[:, k * 128:(k + 1) * 128],
                                     y_sb[:, kc, msl], wv_sb[:, kc, :],
                                     start=(b == mb and kc == 0),
                                     stop=(b == mb + 1 and kc == 3))
            for k, b in enumerate((mb, mb + 1)):
                nc.vector.tensor_copy(
                    v2[:, b, :].rearrange("p (g c) -> p g c", g=2)[:, :, 1:65],
                    ps[:, k * 128:(k + 1) * 128].rearrange(
                        "p (g c) -> p g c", g=2))

        def proj256(dst, w_sb, src, h, name):
            # 256-wide projection so work starts at half-DMA arrival
            ps = psA.tile([128, 256], F32, tag="blk", name=name)
            sl = slice(h * 256, (h + 1) * 256)
            for kc in range(4):
                nc.tensor.matmul(ps, w_sb[:, kc, :], src[:, kc, sl],
                                 start=(kc == 0), stop=(kc == 3))
            nc.vector.tensor_copy(dst[:, sl], ps)

        # ---- prologue: only what gates scores(chunk0, m=0..3); the qT
        # copies go ahead of the v2 copies on the DVE queue since the
        # first exp gates on qT while v2 is only needed two steps in ----
        P00 = None
        if cfg["m0split"]:
            P00 = ppool.tile([128, 1024], BF16, tag="p", name="p0_0")

        def m0_piece(h):
            # tile_position'd matmuls may not share a PSUM bank: head
            # pieces go to separate banks, exp reads them strided.
            nh = slice(h * 256, (h + 1) * 256)
            blk = psA.tile([128, 2, 512], F32, tag="blk", name=f"blk0_0{h}")
            nc.tensor.matmul(blk[:, 0, 0:256], kT[0:64, 0:128],
                             qT[0:64, nh], start=True, stop=True,
                             tile_position=(0, 0))
            nc.tensor.matmul(blk[:, 1, 0:256], kT[64:128, 0:128],
                             qT[64:128, nh], start=True, stop=True,
                             tile_position=(64, 0))
            nc.scalar.activation(P00[:, h * 512:(h + 1) * 512],
                                 blk[:, :, 0:256], EXP, scale=32.0)

        proj256(kT, wk_sb, y_sb, 0, "psk0a")
        proj256(qT, wq_sb, x_sb, 0, "psq0a")
        if cfg["m0split"]:
            m0_piece(0)
        proj256(kT, wk_sb, y_sb, 1, "psk0b")
        proj256(qT, wq_sb, x_sb, 1, "psq0b")
        if cfg["m0split"]:
            m0_piece(1)
        P01 = None
        if cfg["m1early"]:
            # m=1 scores+exp also ahead of the v2 prologue block so the
            # exp stream has no gap while v2 builds
            blk01 = psA.tile([128, 1024], F32, tag="blk", name="blk0_1")
            nc.tensor.matmul(blk01[:, 0:512], kT[0:64, 128:256],
                             qT[0:64, 0:512], start=True, stop=True,
                             tile_position=(0, 0))
            nc.tensor.matmul(blk01[:, 512:1024], kT[64:128, 128:256],
                             qT[64:128, 0:512], start=True, stop=True,
                             tile_position=(64, 0))
            P01 = ppool.tile([128, 1024], BF16, tag="p", name="p0_1")
            nc.scalar.activation(P01, blk01, EXP, scale=32.0)
        # early v2 blocks ride the PE while later DMAs are in flight
        for _mb in range(cfg["v2pro"]):
            v2_task(_mb)

        # weave tasks: late projections + v2 blocks. Emission order defines
        # data dependencies (a consumer emitted before its producer reads
        # stale SBUF), so each task must be emitted strictly before its
        # first consumer: kT j before scores(0, 4j), v2(mb) before the
        # attnout(mb) emission (mb+2), qT j before scores(j, 0).
        def P_(dst, w, src, j, half, name):
            return lambda: proj_half(dst, w, src, j, half, name)

        def proj_q(dst, w_sb, src, j, kc, name):
            if kc == 0:
                hold[name] = psA.tile([128, 512], F32, tag="blk", name=name)
            ps = hold[name]
            nc.tensor.matmul(ps, w_sb[:, kc, :],
                             src[:, kc, j * 512:(j + 1) * 512],
                             start=(kc == 0), stop=(kc == 3))
            if kc == 3:
                nc.vector.tensor_copy(dst[:, j * 512:(j + 1) * 512], ps)

        def Q_(dst, w, src, j, kc, name):
            return lambda: proj_q(dst, w, src, j, kc, name)

        _v2rem = list(range(cfg["v2pro"], 4))
        _v2h = (len(_v2rem) + 1) // 2
        _vs = cfg["v2shift"]
        fills = {
            (0, 0): [(lambda mb=mb: v2_task(mb)) for mb in _v2rem[:_v2h]],
            (0, 1): [(lambda mb=mb: v2_task(mb)) for mb in _v2rem[_v2h:]],
            (0, 2): [P_(kT, wk_sb, y_sb, 1, 0, "psk1")],
            (0, 3): [P_(kT, wk_sb, y_sb, 1, 1, "psk1")],
        }
        _qo = cfg["qoff"]
        fills[(1, _qo)] = [P_(qT, wq_sb, x_sb, 2, 0, "psq2")]
        fills[(1, _qo + 1)] = [P_(qT, wq_sb, x_sb, 2, 1, "psq2")]
        fills[(2, _qo)] = [P_(qT, wq_sb, x_sb, 3, 0, "psq3")]
        fills[(2, _qo + 1)] = [P_(qT, wq_sb, x_sb, 3, 1, "psq3")]
        if cfg["ktq"]:
            # quarter-granular late projections: one matmul per weave slot
            for i, (j, kc) in enumerate([(2, 0), (2, 1), (2, 2), (2, 3),
                                         (3, 0), (3, 1), (3, 2), (3, 3)]):
                fills.setdefault((0, 4 + i), []).append(
                    Q_(kT, wk_sb, y_sb, j, kc, f"psk{j}"))
            for kc in range(4):
                fills.setdefault((0, 12 + kc), []).append(
                    Q_(qT, wq_sb, x_sb, 1, kc, "psq1"))
        else:
            fills[(0, 6)] = [P_(kT, wk_sb, y_sb, 2, 0, "psk2")]
            fills[(0, 7)] = [P_(kT, wk_sb, y_sb, 2, 1, "psk2")]
            fills[(0, 10)] = [P_(kT, wk_sb, y_sb, 3, 0, "psk3")]
            fills[(0, 11)] = [P_(kT, wk_sb, y_sb, 3, 1, "psk3")]
            fills[(0, 13)] = [P_(qT, wq_sb, x_sb, 1, 0, "psq1")]
            fills[(0, 14)] = [P_(qT, wq_sb, x_sb, 1, 1, "psq1")]
        if cfg["v2pair"]:
            # pairs on proj-free steps; each pair's deadline is the attnout
            # pop of its first block (step mb+lag)
            for mb, step in ((4, 4), (6, 5), (8, 8), (10, 9), (12, 12),
                             (14, 15)):
                fills.setdefault((0, step), []).append(
                    lambda mb=mb: v2_task2(mb))
        else:
            for mb in range(4, 16):
                # v2(mb) must be emitted before attnout(mb) pops at step
                # mb+lag (which may fall in chunk 1); shift within that
                # window to smooth the chunk-0 PE load
                step = mb + _vs + (cfg["v2tail"] if mb >= 14 else 0)
                key = (0, step) if step <= 15 else (1, step - 16)
                fills.setdefault(key, []).append(lambda mb=mb: v2_task(mb))

        # ---- main attention loop ----
        pending = []         # (P, acc_a, acc_b, m) awaiting attnout
        post = []            # deferred post-processing closures

        def emit_attnout(P, acc_a, acc_b, m):
            # PSUM zero-region = one full 2KB bank: exactly one start
            # (m=0,s=0) and one stop (m=15,s=3) per accumulator tile.
            for s in range(4):
                if isinstance(P, tuple):  # split first m-step: [a256|b256]x2
                    Pt = P[1]
                    h, i = s // 2, s % 2
                    pa = Pt[:, h * 512 + i * 128:h * 512 + (i + 1) * 128]
                    pb = Pt[:, h * 512 + 256 + i * 128:
                            h * 512 + 256 + (i + 1) * 128]
                else:
                    pa = P[:, s * 128:(s + 1) * 128]
                    pb = P[:, 512 + s * 128:512 + (s + 1) * 128]
                nc.tensor.matmul(acc_a[:, s, :], pa, v2[:, m, 0:65],
                                 start=(m == 0 and s == 0),
                                 stop=(m == 15 and s == 3))
                nc.tensor.matmul(acc_b[:, s, :], pb, v2[:, m, 65:130],
                                 start=(m == 0 and s == 0),
                                 stop=(m == 15 and s == 3))

        def make_post(c, acc_a, acc_b):
            st = {}

            def grab(aps, dst_tag, out_name):
                # one fast PSUM->SBUF copy releases the accumulator bank
                t = spool.tile([128, 4, 65], F32, tag=dst_tag, name=out_name)
                nc.vector.tensor_copy(t, aps)
                return t

            def grab_a():
                st["ca"] = grab(acc_a, "ca", f"ca{c}")
                st["cb"] = grab(acc_b, "cb", f"cb{c}")

            def recips():
                st["attnT"] = npool.tile([128, 512], BF16, tag="attnT",
                                         name=f"attnT{c}")
                if not cfg["divide"]:
                    st["ra"] = spool.tile([128, 4], F32, tag="ra",
                                          name=f"ra{c}")
                    st["rb"] = spool.tile([128, 4], F32, tag="rb",
                                          name=f"rb{c}")
                    nc.vector.reciprocal(st["ra"], st["ca"][:, :, 0:1])
                    nc.vector.reciprocal(st["rb"], st["cb"][:, :, 0:1])

            def norm():
                # both heads' normalize in two broadcast ops
                st["trin"] = spool.tile([128, 4, 128], BF16, tag="trin",
                                        name=f"trin{c}")
                if cfg["divide"]:
                    da = st["ca"][:, :, 0:1].broadcast_to([128, 4, 64])
                    db = st["cb"][:, :, 0:1].broadcast_to([128, 4, 64])
                    nc.vector.tensor_tensor(out=st["trin"][:, :, 0:64],
                                            in0=st["ca"][:, :, 1:65],
                                            in1=da, op=DIV)
                    nc.vector.tensor_tensor(out=st["trin"][:, :, 64:128],
                                            in0=st["cb"][:, :, 1:65],
                                            in1=db, op=DIV)
                else:
                    ra_b = st["ra"][:, :, None].broadcast_to([128, 4, 64])
                    rb_b = st["rb"][:, :, None].broadcast_to([128, 4, 64])
                    nc.vector.tensor_tensor(out=st["trin"][:, :, 0:64],
                                            in0=st["ca"][:, :, 1:65],
                                            in1=ra_b, op=MULT)
                    nc.vector.tensor_tensor(out=st["trin"][:, :, 64:128],
                                            in0=st["cb"][:, :, 1:65],
                                            in1=rb_b, op=MULT)

            def sub(s):
                # paired transposes share one PSUM bank (start zeroes it,
                # second adds into the disjoint half) and one wide copy
                tp = psA.tile([128, 2, 128], BF16, tag="blk",
                              name=f"tp{c}_{s}")
                for k in (0, 1):
                    nc.tensor.matmul(tp[:, k, :], st["trin"][:, s + k, :],
                                     identb, is_transpose=True,
                                     start=(k == 0), stop=(k == 1))
                nc.vector.tensor_copy(
                    st["attnT"][:, s * 128:(s + 2) * 128]
                    .rearrange("p (a b) -> p a b", a=2), tp)

            def outproj(cc):
                po = psA.tile([128, 512], F32, tag="blk", name=f"po{c}_{cc}")
                nc.tensor.matmul(po, wp_sb[:, cc * 128:(cc + 1) * 128],
                                 st["attnT"], start=True, stop=True)
                if cc == 0:
                    st["so"] = npool.tile([128, 4, 512], BF16, tag="so",
                                          name=f"so{c}")
                if cfg["so_act"] and cc % 2 == 1:
                    nc.scalar.copy(st["so"][:, cc, :], po)
                else:
                    nc.vector.tensor_copy(st["so"][:, cc, :], po)
                if cc == 3:
                    nc.sync.dma_start(
                        out=outT4[:, :, c * 512:(c + 1) * 512],
                        in_=st["so"])

            tasks = ([[grab_a], [recips], [norm]]
                     + [[lambda s=s: sub(s)] for s in (0, 2)]
                     + [[lambda cc=cc: outproj(cc)] for cc in range(4)])
            return tasks

        for c in range(4):
            ns = slice(c * 512, (c + 1) * 512)
            acc_a = psB.tile([128, 4, 65], F32, tag="acc", name=f"acca{c}")
            acc_b = psB.tile([128, 4, 65], F32, tag="acc", name=f"accb{c}")
            sched = None
            if cfg["post_steps"] is not None and post:
                sched = {}
                for i, grp in enumerate(post):
                    sched.setdefault(cfg["post_steps"][i], []).append(grp)
                post = []
            for m in range(16):
                ms = slice(m * 128, (m + 1) * 128)
                if cfg["m0split"] and c == 0 and m == 0:
                    # scores+exp for m0 were already emitted in the
                    # prologue (split into two x-half-gated pieces with
                    # layout [a n256 | b n256] per half)
                    P = ("split", P00)
                elif cfg["m1early"] and c == 0 and m == 1:
                    P = P01
                else:
                    blk = psA.tile([128, 1024], F32, tag="blk",
                                   name=f"blk{c}_{m}")
                    nc.tensor.matmul(blk[:, 0:512], kT[0:64, ms],
                                     qT[0:64, ns],
                                     start=True, stop=True,
                                     tile_position=(0, 0))
                    nc.tensor.matmul(blk[:, 512:1024], kT[64:128, ms],
                                     qT[64:128, ns],
                                     start=True, stop=True,
                                     tile_position=(64, 0))
                    P = ppool.tile([128, 1024], BF16, tag="p",
                                   name=f"p{c}_{m}")
                    dn = dvexp.get((c, m), 0)
                    if dn:
                        # split the exp: ScalarE takes cols [0:1024-dn],
                        # DVE (poly + 5 squarings) takes the tail [1024-dn:]
                        x0 = 1024 - dn
                        if x0:
                            nc.scalar.activation(P[:, 0:x0], blk[:, 0:x0],
                                                 EXP, scale=32.0)
                        q = qpool.tile([128, dn], F32, tag="q",
                                       name=f"q{c}_{m}")
                        nc.vector._custom_dve(EXP_POLY, out=q,
                                              in0=blk[:, x0:1024],
                                              s0=EXP_C0, s1=EXP_C1,
                                              imm2=EXP_C2)
                        nc.vector._custom_dve(EXP_SQ32, out=P[:, x0:1024],
                                              in0=q)
                    else:
                        nc.scalar.activation(P, blk, EXP, scale=32.0)
                if sched is not None:
                    for grp in sched.pop(m, ()):
                        for task in grp:
                            task()
                elif post and m >= cfg["post0"]:
                    for task in post.pop(0):
                        task()
                for task in fills.pop((c, m), ()):
                    task()
                pending.append((P, acc_a, acc_b, m))
                # lag 3 mid-chunk; at m=14/15 drain the backlog down to 1
                # (PE has slack there) so chunk boundaries don't pile
                # attnouts on top of the next chunk's first scores; a new
                # chunk's first attnout (which waits for the previous
                # accumulator bank to be copied out by grab_a) is held until
                # m=3; the last chunk defers its final steps entirely.
                lag_now = cfg["lag"] if m < 14 else (
                    cfg["endlag"] if m == 14 else cfg["endlag"] - 1)
                while len(pending) > lag_now and not (
                        c > 0 and m < cfg["hold"] and
                        pending[0][3] == 0) and not (
                        cfg["defer"] and c == 3 and m >= cfg["defer_m"]):
                    emit_attnout(*pending.pop(0))
            post = make_post(c, acc_a, acc_b)

        # ---- drain + tail (chunk 3 post-processing, pipelined) ----
        # Read the accumulators straight from PSUM (no ring pressure at
        # the end), split normalize/copy work across DVE and the now-idle
        # ScalarE (Copy shares the exp activation table, no reload), and
        # run the output projection per n-sub-block so copies and DMAs
        # start as early as possible. Two po tiles reuse the accumulator
        # banks freed at the start of the tail.
        while pending:
            emit_attnout(*pending.pop(0))
        c = 3
        # normalize both heads straight from PSUM with two broadcast
        # divides (no reciprocal step on the tail critical path)
        trin3 = spool.tile([128, 4, 128], BF16, tag="trin", name="trin3")
        if cfg["divide_tail"]:
            nc.vector.tensor_tensor(
                out=trin3[:, :, 0:64], in0=acc_a[:, :, 1:65],
                in1=acc_a[:, :, 0:1].broadcast_to([128, 4, 64]), op=DIV)
            nc.vector.tensor_tensor(
                out=trin3[:, :, 64:128], in0=acc_b[:, :, 1:65],
                in1=acc_b[:, :, 0:1].broadcast_to([128, 4, 64]), op=DIV)
        else:
            ra = spool.tile([128, 4], F32, tag="ra", name="ra3")
            rb = spool.tile([128, 4], F32, tag="rb", name="rb3")
            nc.vector.reciprocal(ra, acc_a[:, :, 0:1])
            nc.vector.reciprocal(rb, acc_b[:, :, 0:1])
            nc.vector.tensor_tensor(
                out=trin3[:, :, 0:64], in0=acc_a[:, :, 1:65],
                in1=ra[:, :, None].broadcast_to([128, 4, 64]), op=MULT)
            nc.vector.tensor_tensor(
                out=trin3[:, :, 64:128], in0=acc_b[:, :, 1:65],
                in1=rb[:, :, None].broadcast_to([128, 4, 64]), op=MULT)
        po = [psB.tile([128, 512], F32, tag="acc", name="po3_0"),
              psB.tile([128, 512], F32, tag="acc", name="po3_1"),
              psA.tile([128, 512], F32, tag="blk", name="po3_2"),
              psA.tile([128, 512], F32, tag="blk", name="po3_3")]
        # paired transposes -> two half-attnT tiles with one writer each so
        # the PSUM->SBUF copies run concurrently on DVE and ScalarE
        attnTh = [npool.tile([128, 2, 128], BF16, tag=f"attnT3_{h}",
                             name=f"attnT3_{h}") for h in (0, 1)]
        for h in (0, 1):
            tp = psA.tile([128, 2, 128], BF16, tag="blk", name=f"tp3_{h}")
            for k in (0, 1):
                nc.tensor.matmul(tp[:, k, :], trin3[:, 2 * h + k, :],
                                 identb, is_transpose=True,
                                 start=(k == 0), stop=(k == 1))
            if h == 0:
                nc.vector.tensor_copy(attnTh[h], tp)
            else:
                nc.scalar.copy(attnTh[h], tp)
        for s in range(4):
            for cc in range(4):
                nc.tensor.matmul(po[cc][:, s * 128:(s + 1) * 128],
                                 wp_sb[:, cc * 128:(cc + 1) * 128],
                                 attnTh[s // 2][:, s % 2, :],
                                 start=(s == 0), stop=(s == 3))
        so3 = npool.tile([128, 4, 512], BF16, tag="so", name="so3")
        nc.vector.tensor_copy(so3[:, 0, :], po[0])
        nc.scalar.copy(so3[:, 1, :], po[1])
        nc.sync.dma_start(out=outT4[:, 0:2, c * 512:(c + 1) * 512],
                          in_=so3[:, 0:2, :])
        nc.vector.tensor_copy(so3[:, 2, :], po[2])
        nc.scalar.copy(so3[:, 3, :], po[3])
        nc.sync.dma_start(out=outT4[:, 2:4, c * 512:(c + 1) * 512],
                          in_=so3[:, 2:4, :])

    nc.compile()
    return nc


def _get_program():
    global _NC
    if _NC is None:
        _NC = _build_program()
    return _NC


def make_in_maps(inputs):
    import ml_dtypes
    bf16 = ml_dtypes.bfloat16

    x = np.asarray(inputs["x"], np.float32)
    y = np.asarray(inputs["y"], np.float32)
    Wq = np.asarray(inputs["Wq"], np.float32)
    Wkv = np.asarray(inputs["Wkv"], np.float32)
    lw = np.asarray(inputs["lw"], np.float32)
    Wp = np.asarray(inputs["Wp"], np.float32)

    d = np.arange(HD)
    xb = [np.ascontiguousarray(x[b]).astype(bf16) for b in range(B)]
    yb = [np.ascontiguousarray(y[b]).astype(bf16) for b in range(B)]
    in_maps = []
    for core in range(NCORES):
        b = core // 4
        h0 = (core % 4) * 2
        ch = np.concatenate([h * HD + d for h in (h0, h0 + 1)])  # channels
        colsK = np.concatenate([h * 2 * HD + 2 * d for h in (h0, h0 + 1)])
        wq_c = Wq[:, ch] * np.float32(SCALE / 32.0)
        wk_c = Wkv[:, colsK]
        wv_c = Wkv[:, colsK + 1] * (1.0 + lw[ch])[None, :]
        wp_c = Wp[ch, :]
        wall = np.concatenate([wk_c, wq_c, wv_c], axis=1)  # [C, 384]
        in_maps.append({
            "xr": xb[b],
            "yr": yb[b],
            "wall": np.ascontiguousarray(wall).astype(bf16),
            "wp": np.ascontiguousarray(wp_c).astype(bf16),
        })
    return in_maps


def assemble_output(results, inputs):
    lb = np.asarray(inputs["lb"], np.float32)
    Wp = np.asarray(inputs["Wp"], np.float32)
    bp = np.asarray(inputs["bp"], np.float32)
    bias = (bp + lb @ Wp).astype(np.float32)
    parts = [np.asarray(results[i]["outT"], dtype=np.float32)
             for i in range(NCORES)]
    out = np.stack([
        parts[0] + parts[1] + parts[2] + parts[3],
        parts[4] + parts[5] + parts[6] + parts[7],
    ])
    out += bias[None, :, None]
    return out.astype(np.float32)


def kernel(x, y, Wq, Wkv, lw, lb, Wp, bp):
    global LAST_RUN
    from concourse.bass_utils import run_bass_kernel_spmd

    inputs = dict(x=x, y=y, Wq=Wq, Wkv=Wkv, lw=lw, lb=lb, Wp=Wp, bp=bp)
    nc = _get_program()
    in_maps = make_in_maps(inputs)
    LAST_RUN = run_bass_kernel_spmd(nc, in_maps, list(range(NCORES)))
    return assemble_output(LAST_RUN.results, inputs)

